# revision 1
# baseline (speedup 1.0000x reference)
"""Trainium2 Bass kernel for an encoder-decoder (S2S) transformer.

Distribution: 8 NeuronCores = 4 data-parallel groups (batch B=4) x 2-way
Megatron tensor-parallel within each same-SEngine core pair.  Per TP pair,
attention heads (qkv/out) and ffn (ff1/ff2) are sharded; partials combine
with a 2-core AllReduce after the attention out-projection and after ff2.

Matmuls run in bf16 on the TensorEngine (fp32 PSUM accumulation); the
residual stream and layernorm statistics stay fp32.  Activations are
SBUF-resident for the whole pass; only weights stream from HBM.

Layout conventions (per core, P=128):
  x_nat[t]  [P, D]   fp32   natural tokens-on-partitions residual stream
  x_T       [P, ND*T] bf16  transposed: chunk d cols [d*T:(d+1)*T]
  q_T/k_T   [P, NQK*T] bf16 rows = sharded head dims, chunk m = heads 2m,2m+1
  v_nat     [P, NT*DL] bf16 t-chunk cols [t*DL:(t+1)*DL]
  attn_T    [P, (DL/P)*T] bf16  context, transposed (rows = sharded dims)
  h_T       [P, NFF*T] bf16 ffn hidden, transposed
"""

import os
import sys

for _p in ("/opt/trn_rl_repo", "/root/.axon_site/_ro/trn_rl_repo"):
    if os.path.isdir(_p) and _p not in sys.path:
        sys.path.insert(0, _p)

import numpy as np
import ml_dtypes

import concourse.bass as bass
import concourse.bacc as bacc
import concourse.tile as tile
from concourse import mybir
from concourse.bass import IndirectOffsetOnAxis
from concourse.masks import make_identity, make_causal_mask

BF16 = ml_dtypes.bfloat16
F32 = mybir.dt.float32
BF = mybir.dt.bfloat16
I32 = mybir.dt.int32
AF = mybir.ActivationFunctionType
ALU = mybir.AluOpType
AX = mybir.AxisListType

P = 128


class Cfg:
    def __init__(self, B, Q, T, D, H, V, L, FF, TP, n_cores, flags=frozenset()):
        self.B, self.Q, self.T, self.D, self.H, self.V, self.L, self.FF = \
            B, Q, T, D, H, V, L, FF
        self.TP, self.n_cores = TP, n_cores
        self.E = D // Q
        self.HD = D // H
        assert self.HD == 64, "head packing assumes head_dim 64"
        assert self.E == P, "per-quantizer embedding dim must be 128"
        self.SCALE = 1.0 / float(np.sqrt(self.HD))
        self.DL = D // TP
        self.FFL = FF // TP
        self.HL = H // TP
        self.NT = T // P
        self.ND = D // P
        self.NQK = self.DL // P
        self.NO = self.DL // P      # attn_T chunks
        self.NFF = self.FFL // P
        assert self.HL % 2 == 0, "needs an even number of local heads"
        self.flags = frozenset(flags)

    def key(self):
        return (self.B, self.Q, self.T, self.D, self.H, self.V, self.L,
                self.FF, self.TP, self.n_cores, tuple(sorted(self.flags)))


# --------------------------------------------------------------------------
# program builder
# --------------------------------------------------------------------------

def build_program(c: Cfg):
    nc = bacc.Bacc(None, target_bir_lowering=False, num_devices=c.n_cores)

    def din(name, shape, dt=BF):
        return nc.dram_tensor(name, shape, dt, kind="ExternalInput")

    codes_in = din("codes_in", [c.Q, c.T], I32)
    codes_tgt = din("codes_tgt", [c.Q, c.T], I32)
    tok_emb = [din(f"tok_emb_{q}", [c.V, c.E], F32) for q in range(c.Q)]
    pos = din("pos", [c.T, c.D], F32)

    w = {}
    for l in range(c.L):
        for nm, sh in [
                (f"e_qkv_{l}", [c.D, 3 * c.DL]), (f"e_out_{l}", [c.DL, c.D]),
                (f"e_ff1_{l}", [c.D, c.FFL]), (f"e_ff2_{l}", [c.FFL, c.D]),
                (f"d_sqkv_{l}", [c.D, 3 * c.DL]), (f"d_sout_{l}", [c.DL, c.D]),
                (f"d_cqkv_{l}", [c.D, 3 * c.DL]), (f"d_cout_{l}", [c.DL, c.D]),
                (f"d_ff1_{l}", [c.D, c.FFL]), (f"d_ff2_{l}", [c.FFL, c.D])]:
            w[nm] = din(nm, sh)
    w["head_t"] = din("head_t", [c.E, c.Q * c.V])

    opt = {}
    for nm in c.flags:
        if "_qkv_b_" in nm or "_sqkv_b_" in nm or "_cqkv_b_" in nm:
            opt[nm] = din(nm, [3 * c.DL], F32)
        elif "_ff1_b_" in nm:
            opt[nm] = din(nm, [c.FFL], F32)
        elif nm == "head_b":
            opt[nm] = din(nm, [P, c.Q * c.V], F32)
        else:
            opt[nm] = din(nm, [P, c.D], F32)

    logits = nc.dram_tensor("logits", [c.Q, c.T, c.V], F32,
                            kind="ExternalOutput")

    groups = ([[g * c.TP + i for i in range(c.TP)]
               for g in range(c.n_cores // c.TP)] if c.TP > 1 else None)

    with tile.TileContext(nc) as tc:
        _emit(nc, tc, c, codes_in, codes_tgt, tok_emb, pos, w, opt, logits,
              groups)
    nc.compile()
    return nc


def _emit(nc, tc, c, codes_in, codes_tgt, tok_emb, pos, w, opt, logits,
          groups):
    from contextlib import ExitStack
    es = ExitStack()
    pool = lambda name, bufs, space="SBUF": es.enter_context(
        tc.tile_pool(name=name, bufs=bufs, space=space))

    const = pool("const", 1)
    persist = pool("persist", 1)
    wpool = pool("wpool", 3)
    act = pool("act", 3)
    scratch = pool("scratch", 2)
    dram = pool("dram", 4, space="DRAM")
    ps_proj = pool("ps_proj", 2, space="PSUM")
    ps_s = pool("ps_s", 2, space="PSUM")
    ps_av = pool("ps_av", 2, space="PSUM")
    ps_tr = pool("ps_tr", 2, space="PSUM")

    # constants
    ident_f = const.tile([P, P], F32, name="ident_f")
    make_identity(nc, ident_f[:])
    causT = const.tile([P, P], F32, name="causT")
    nc.gpsimd.memset(causT[:], 0.0)
    # transposed causal: fill -1e9 where tk > tq  (iota = tq - tk >= 0 keeps)
    nc.gpsimd.affine_select(out=causT[:], in_=causT[:],
                            compare_op=ALU.is_ge, fill=-1e9, base=0,
                            pattern=[[1, P]], channel_multiplier=-1)

    eps_t = const.tile([P, 1], F32, name="eps_t")
    nc.vector.memset(eps_t[:], 1e-5)

    opt_sb = {}
    for nm in opt:
        if "qkv_b_" in nm:
            t = const.tile([P, 3 * c.NQK], F32, name=f"sb_{nm}")
            nc.sync.dma_start(out=t[:],
                              in_=opt[nm].rearrange("(m p) -> p m", p=P))
        elif "_ff1_b_" in nm:
            t = const.tile([P, c.NFF], F32, name=f"sb_{nm}")
            nc.sync.dma_start(out=t[:],
                              in_=opt[nm].rearrange("(m p) -> p m", p=P))
        else:
            sh = [P, c.Q * c.V] if nm == "head_b" else [P, c.D]
            t = const.tile(sh, F32, name=f"sb_{nm}")
            nc.sync.dma_start(out=t[:], in_=opt[nm][:])
        opt_sb[nm] = t

    # persistent activations
    x_nat = [persist.tile([P, c.D], F32, name=f"x_{t}", tag=f"x_{t}")
             for t in range(c.NT)]
    x_T = persist.tile([P, c.ND * c.T], BF, name="x_T", tag="x_T")
    mem_T = persist.tile([P, c.ND * c.T], BF, name="mem_T", tag="mem_T")
    q_T = persist.tile([P, c.NQK * c.T], BF, name="q_T", tag="q_T")
    k_T = persist.tile([P, c.NQK * c.T], BF, name="k_T", tag="k_T")
    WA = c.HL * 128   # per-t block: per head 64 value cols + 64 ones cols
    v_aug = persist.tile([P, c.NT * WA], BF, name="v_aug", tag="v_aug")
    attn_T = persist.tile([P, c.NO * c.T], BF, name="attn_T", tag="attn_T")
    h_T = persist.tile([P, c.NFF * c.T], BF, name="h_T", tag="h_T")

    # ---------------- helpers ----------------
    def transpose_x_into(dst, copy_eng):
        """dst[:, d*T + t*P : +P] = x_nat[t][:, d*P:+P].T (fp32 -> bf16)."""
        for t in range(c.NT):
            for d in range(c.ND):
                pt = ps_tr.tile([P, P], F32, tag="ps_tr", name="pt")
                nc.tensor.transpose(pt[:], x_nat[t][:, d * P:(d + 1) * P],
                                    ident_f[:])
                dsl = dst[:, d * c.T + t * P: d * c.T + t * P + P]
                if (t + d) % 2 == 0:
                    nc.scalar.copy(out=dsl, in_=pt[:])
                else:
                    nc.vector.tensor_copy(out=dsl, in_=pt[:])

    def embed(codes):
        for t in range(c.NT):
            ptile = scratch.tile([P, c.D], F32, tag="pos", name="ptile")
            nc.sync.dma_start(out=ptile[:], in_=pos[t * P:(t + 1) * P, :])
            for q in range(c.Q):
                idx = scratch.tile([P, 1], I32, tag="idx", name="idx")
                nc.sync.dma_start(out=idx[:], in_=codes[q, t * P:(t + 1) * P])
                nc.gpsimd.indirect_dma_start(
                    out=x_nat[t][:, q * c.E:(q + 1) * c.E],
                    out_offset=None,
                    in_=tok_emb[q][:],
                    in_offset=IndirectOffsetOnAxis(ap=idx[:, :1], axis=0))
            nc.vector.tensor_tensor(out=x_nat[t][:], in0=x_nat[t][:],
                                    in1=ptile[:], op=ALU.add)

    def ln_tile(t, sub, gname, bname):
        """x_nat[t] = LN(x_nat[t] + sub) * g + b (post-norm)."""
        xt = x_nat[t]
        s1 = scratch.tile([P, 1], F32, tag="lnstat", name="s1", bufs=8)
        s2 = scratch.tile([P, 1], F32, tag="lnstat", name="s2", bufs=8)
        sq = scratch.tile([P, c.D], BF, tag="lnsq", name="sq")
        nc.vector.tensor_tensor(out=xt[:], in0=xt[:], in1=sub[:], op=ALU.add)
        nc.vector.reduce_sum(out=s1[:], in_=xt[:], axis=AX.X)
        nc.scalar.activation(sq[:], xt[:], AF.Square, accum_out=s2[:])
        mean = scratch.tile([P, 1], F32, tag="lnstat", name="mean", bufs=8)
        var = scratch.tile([P, 1], F32, tag="lnstat", name="var", bufs=8)
        m2 = scratch.tile([P, 1], F32, tag="lnstat", name="m2", bufs=8)
        nc.vector.tensor_scalar_mul(mean[:], s1[:], 1.0 / c.D)
        nc.vector.tensor_scalar_mul(var[:], s2[:], 1.0 / c.D)
        nc.vector.tensor_tensor(out=m2[:], in0=mean[:], in1=mean[:],
                                op=ALU.mult)
        nc.vector.tensor_tensor(out=var[:], in0=var[:], in1=m2[:],
                                op=ALU.subtract)
        rstd = scratch.tile([P, 1], F32, tag="lnstat", name="rstd", bufs=8)
        nc.scalar.activation(rstd[:], var[:], AF.Sqrt, bias=eps_t[:])
        nc.vector.reciprocal(rstd[:], rstd[:])
        nc.vector.tensor_scalar(out=xt[:], in0=xt[:], scalar1=mean[:],
                                scalar2=rstd[:], op0=ALU.subtract,
                                op1=ALU.mult)
        if gname in opt_sb:
            nc.vector.tensor_tensor(out=xt[:], in0=xt[:],
                                    in1=opt_sb[gname][:], op=ALU.mult)
        if bname in opt_sb:
            nc.vector.tensor_tensor(out=xt[:], in0=xt[:],
                                    in1=opt_sb[bname][:], op=ALU.add)

    def ar_ln(parts, gname, bname):
        """AllReduce sublayer partials over the TP pair (split in two
        pipelined halves), then per-tile residual add + layernorm."""
        if groups is None:
            for t in range(c.NT):
                ln_tile(t, parts[t], gname, bname)
            return
        NH = max(c.NT // 2, 1)
        for h0 in range(0, c.NT, NH):
            n = min(NH, c.NT - h0)
            arin = dram.tile([n * P, c.D], BF, tag="arin", name="arin")
            arout = dram.tile([n * P, c.D], BF, tag="arout", name="arout")
            for i in range(n):
                nc.sync.dma_start(out=arin[i * P:(i + 1) * P, :],
                                  in_=parts[h0 + i][:])
            nc.gpsimd.collective_compute(
                "AllReduce", ALU.add, replica_groups=groups,
                ins=[arin[:].opt()], outs=[arout[:].opt()])
            for i in range(n):
                red = scratch.tile([P, c.D], BF, tag="ar_red", name="red")
                nc.sync.dma_start(out=red[:], in_=arout[i * P:(i + 1) * P, :])
                ln_tile(h0 + i, red, gname, bname)

    def proj_rows(wname, bname, src_T, row_base, dst):
        """dst[:, m*T:(m+1)*T] rows [row_base + m*P ...] of W.T @ src."""
        for m in range(c.NQK):
            mg = row_base // P + m
            wt = wpool.tile([P, c.ND * P], BF, tag="wqkv", name="wt")
            nc.sync.dma_start(
                out=wt[:].rearrange("p (nd m) -> p nd m", m=P),
                in_=w[wname][:, row_base + m * P: row_base + (m + 1) * P]
                .rearrange("(nd p) m -> p nd m", p=P))
            ps = ps_proj.tile([P, c.T], F32, tag="ps_proj", name="ps")
            for k in range(c.ND):
                nc.tensor.matmul(ps[:], wt[:, k * P:(k + 1) * P],
                                 src_T[:, k * c.T:(k + 1) * c.T],
                                 start=(k == 0), stop=(k == c.ND - 1))
            col = m * c.T
            if bname in opt_sb:
                nc.vector.tensor_scalar(
                    out=dst[:, col:col + c.T], in0=ps[:],
                    scalar1=opt_sb[bname][:, mg:mg + 1], scalar2=None,
                    op0=ALU.add)
            else:
                nc.scalar.copy(out=dst[:, col:col + c.T], in_=ps[:])

    def proj_v(wname, bname, src_T):
        wts = []
        for k in range(c.ND):
            wt = wpool.tile([P, c.DL], BF, tag=f"wv_{k}", name="wt", bufs=1)
            nc.sync.dma_start(
                out=wt[:],
                in_=w[wname][k * P:(k + 1) * P, 2 * c.DL:3 * c.DL])
            wts.append(wt)
        for t in range(c.NT):
            ps = ps_proj.tile([P, c.DL], F32, tag="ps_proj", name="ps")
            for k in range(c.ND):
                nc.tensor.matmul(
                    ps[:], src_T[:, k * c.T + t * P: k * c.T + t * P + P],
                    wts[k][:], start=(k == 0), stop=(k == c.ND - 1))
            blk = v_aug[:, t * WA:(t + 1) * WA].rearrange(
                "p (h f) -> p h f", f=128)
            nc.scalar.copy(out=blk[:, :, 0:64],
                           in_=ps[:].rearrange("p (h f) -> p h f", f=64))
            if bname in opt_sb:
                raise NotImplementedError("nonzero v bias unsupported")

    # ones columns of v_aug are set once; projections only write data cols
    for t in range(c.NT):
        blk = v_aug[:, t * WA:(t + 1) * WA].rearrange("p (h f) -> p h f", f=128)
        nc.vector.memset(blk[:, :, 64:128], 1.0)

    def attention(causal):
        """q_T,k_T,v_aug -> attn_T.  Scores are computed pre-transposed
        (S^T[tk, tq] = k_blk^T q) so no A transposes are needed; the softmax
        denominator rides along as a ones column in v_aug and the division
        happens on the small [64, T] context output."""
        for h in range(c.HL):
            m = h // 2
            po = 64 * (h % 2)
            at_tiles = [act.tile([P, c.T], BF, tag=f"AT{tk}", name="at",
                                 bufs=2) for tk in range(c.NT)]
            for tk in range(c.NT):
                tq0 = tk * P if causal else 0
                pss = ps_s.tile([P, c.T], F32, tag="ps_s", name="pss")
                nc.tensor.matmul(
                    pss[:, tq0:c.T],
                    k_T[po:po + 64, m * c.T + tk * P: m * c.T + tk * P + P],
                    q_T[po:po + 64, m * c.T + tq0: m * c.T + c.T],
                    start=True, stop=True)
                if causal:
                    nc.vector.tensor_tensor(
                        out=pss[:, tk * P:(tk + 1) * P],
                        in0=pss[:, tk * P:(tk + 1) * P],
                        in1=causT[:], op=ALU.add)
                nc.scalar.activation(at_tiles[tk][:, tq0:c.T],
                                     pss[:, tq0:c.T], AF.Exp, scale=c.SCALE)
            ps_o = ps_av.tile([P, c.T], F32, tag="ps_av", name="ps_o")
            for tk in range(c.NT):
                cols0 = tk * P if causal else 0
                nc.tensor.matmul(
                    ps_o[:, cols0:c.T],
                    v_aug[:, tk * WA + h * 128: tk * WA + h * 128 + 128],
                    at_tiles[tk][:, cols0:c.T],
                    start=(tk == 0), stop=(tk == c.NT - 1))
            rden = scratch.tile([64, c.T], F32, tag="rden", name="rden",
                                bufs=2)
            nc.vector.reciprocal(rden[:], ps_o[64:128, :])
            nc.vector.tensor_tensor(
                out=attn_T[po:po + 64, m * c.T:(m + 1) * c.T],
                in0=ps_o[0:64, :], in1=rden[:], op=ALU.mult)

    NB = max(c.D // 512, 1)
    NW = min(512, c.D)

    def mm_to_natural(src_T, nk, wts, bname):
        """[T, D] = src_T.T @ W, returned as per-t bf16 [P, D] tiles."""
        parts = []
        for t in range(c.NT):
            sb = scratch.tile([P, c.D], BF, tag="oproj", name="sb", bufs=3)
            for n in range(NB):
                ps = ps_proj.tile([P, NW], F32, tag="ps_proj", name="ps")
                for k in range(nk):
                    nc.tensor.matmul(
                        ps[:], src_T[:, k * c.T + t * P: k * c.T + t * P + P],
                        wts[k][:, n * NW:(n + 1) * NW],
                        start=(k == 0), stop=(k == nk - 1))
                nc.scalar.copy(out=sb[:, n * NW:(n + 1) * NW], in_=ps[:])
            if bname in opt_sb:
                nc.vector.tensor_tensor(out=sb[:], in0=sb[:],
                                        in1=opt_sb[bname][:], op=ALU.add)
            parts.append(sb)
        return parts

    def out_proj(wname, bname):
        wts = []
        for k in range(c.NO):
            wt = wpool.tile([P, c.D], BF, tag=f"wo_{k}", name="wt", bufs=1)
            nc.sync.dma_start(out=wt[:], in_=w[wname][k * P:(k + 1) * P, :])
            wts.append(wt)
        return mm_to_natural(attn_T, c.NO, wts, bname)

    def ffn(w1name, b1name, w2name, b2name):
        for m in range(c.NFF):
            wt = wpool.tile([P, c.ND * P], BF, tag="wff1", name="wt")
            nc.sync.dma_start(
                out=wt[:].rearrange("p (nd m) -> p nd m", m=P),
                in_=w[w1name][:, m * P:(m + 1) * P]
                .rearrange("(nd p) m -> p nd m", p=P))
            ps = ps_proj.tile([P, c.T], F32, tag="ps_proj", name="ps")
            for k in range(c.ND):
                nc.tensor.matmul(ps[:], wt[:, k * P:(k + 1) * P],
                                 x_T[:, k * c.T:(k + 1) * c.T],
                                 start=(k == 0), stop=(k == c.ND - 1))
            if b1name in opt_sb:
                nc.scalar.activation(h_T[:, m * c.T:(m + 1) * c.T], ps[:],
                                     AF.Relu,
                                     bias=opt_sb[b1name][:, m:m + 1])
            else:
                nc.scalar.activation(h_T[:, m * c.T:(m + 1) * c.T], ps[:],
                                     AF.Relu)
        wts = []
        for k in range(c.NFF):
            wt = wpool.tile([P, c.D], BF, tag=f"wff2_{k}", name="wt", bufs=1)
            nc.sync.dma_start(out=wt[:], in_=w[w2name][k * P:(k + 1) * P, :])
            wts.append(wt)
        return mm_to_natural(h_T, c.NFF, wts, b2name)

    # ---------------- encoder ----------------
    embed(codes_in)
    for l in range(c.L):
        transpose_x_into(x_T, nc.vector)
        proj_rows(f"e_qkv_{l}", f"e_qkv_b_{l}", x_T, 0, q_T)
        proj_rows(f"e_qkv_{l}", f"e_qkv_b_{l}", x_T, c.DL, k_T)
        proj_v(f"e_qkv_{l}", None, x_T)
        attention(causal=False)
        ar_ln(out_proj(f"e_out_{l}", f"e_out_b_{l}"), f"e_ln1_w_{l}", f"e_ln1_b_{l}")
        transpose_x_into(x_T, nc.vector)
        ar_ln(
            ffn(f"e_ff1_{l}", f"e_ff1_b_{l}", f"e_ff2_{l}", f"e_ff2_b_{l}"), f"e_ln2_w_{l}", f"e_ln2_b_{l}")
    transpose_x_into(mem_T, nc.scalar)

    # ---------------- decoder ----------------
    embed(codes_tgt)
    for l in range(c.L):
        transpose_x_into(x_T, nc.vector)
        proj_rows(f"d_sqkv_{l}", f"d_sqkv_b_{l}", x_T, 0, q_T)
        proj_rows(f"d_sqkv_{l}", f"d_sqkv_b_{l}", x_T, c.DL, k_T)
        proj_v(f"d_sqkv_{l}", None, x_T)
        attention(causal=True)
        _sp = out_proj(f"d_sout_{l}", f"d_sout_b_{l}")
        # cross-attention K/V depend only on encoder memory -- emitted here
        # so the TensorEngine stays busy while the AllReduce is in flight
        proj_rows(f"d_cqkv_{l}", f"d_cqkv_b_{l}", mem_T, c.DL, k_T)
        proj_v(f"d_cqkv_{l}", None, mem_T)
        ar_ln(_sp, f"d_ln1_w_{l}", f"d_ln1_b_{l}")
        transpose_x_into(x_T, nc.vector)
        proj_rows(f"d_cqkv_{l}", f"d_cqkv_b_{l}", x_T, 0, q_T)
        attention(causal=False)
        ar_ln(out_proj(f"d_cout_{l}", f"d_cout_b_{l}"), f"d_ln2_w_{l}", f"d_ln2_b_{l}")
        transpose_x_into(x_T, nc.vector)
        ar_ln(
            ffn(f"d_ff1_{l}", f"d_ff1_b_{l}", f"d_ff2_{l}", f"d_ff2_b_{l}"), f"d_ln3_w_{l}", f"d_ln3_b_{l}")

    # ---------------- output head (all Q on every core) ----------------
    transpose_x_into(x_T, nc.vector)   # x_T now holds y_T
    NBV = max(c.V // 512, 1)
    NWV = min(512, c.V)
    for j in range(c.Q):
        hw = wpool.tile([P, c.V], BF, tag="whead", name="hw", bufs=2)
        nc.sync.dma_start(out=hw[:],
                          in_=w["head_t"][:, j * c.V:(j + 1) * c.V])
        for t in range(c.NT):
            sb = scratch.tile([P, c.V], F32, tag="lgt", name="sb", bufs=3)
            for n in range(NBV):
                ps = ps_proj.tile([P, NWV], F32, tag="ps_proj", name="ps")
                nc.tensor.matmul(
                    ps[:], x_T[:, j * c.T + t * P: j * c.T + t * P + P],
                    hw[:, n * NWV:(n + 1) * NWV], start=True, stop=True)
                nc.scalar.copy(out=sb[:, n * NWV:(n + 1) * NWV], in_=ps[:])
            if "head_b" in opt_sb:
                nc.vector.tensor_tensor(
                    out=sb[:], in0=sb[:],
                    in1=opt_sb["head_b"][:, j * c.V:(j + 1) * c.V],
                    op=ALU.add)
            nc.sync.dma_start(out=logits[j, t * P:(t + 1) * P, :], in_=sb[:])

    es.close()


# --------------------------------------------------------------------------
# host side
# --------------------------------------------------------------------------

_PROG_CACHE = {}


def parse_cfg(inputs, TP=None, n_cores=None):
    B, Q, T = inputs["input_codes"].shape
    _, V, E = np.asarray(inputs["tok_emb"]).shape
    L, _, D = np.asarray(inputs["e_qkv_w"]).shape
    FF = np.asarray(inputs["e_ff1_w"]).shape[1]
    H = D // 64
    if TP is None:
        TP = int(os.environ.get("BASS_S2S_TP", "2"))
    if n_cores is None:
        n_cores = B * TP
    flags = set()
    for l in range(L):
        for ref, knm in [("e_qkv_b", "e_qkv_b"), ("d_sqkv_b", "d_sqkv_b"),
                         ("d_cqkv_b", "d_cqkv_b"), ("e_ff1_b", "e_ff1_b"),
                         ("d_ff1_b", "d_ff1_b"), ("e_out_b", "e_out_b"),
                         ("e_ff2_b", "e_ff2_b"), ("d_sout_b", "d_sout_b"),
                         ("d_cout_b", "d_cout_b"), ("d_ff2_b", "d_ff2_b")]:
            if np.any(np.asarray(inputs[ref])[l]):
                flags.add(f"{knm}_{l}")
        for ln in ["e_ln1", "e_ln2", "d_ln1", "d_ln2", "d_ln3"]:
            if not np.all(np.asarray(inputs[ln + "_w"])[l] == 1.0):
                flags.add(f"{ln}_w_{l}")
            if np.any(np.asarray(inputs[ln + "_b"])[l]):
                flags.add(f"{ln}_b_{l}")
    if np.any(np.asarray(inputs["head_b"])):
        flags.add("head_b")
    # v-bias unsupported in-kernel; fall back assertion
    for l in range(L):
        for nm in ["e_qkv_b", "d_sqkv_b", "d_cqkv_b"]:
            vb = np.asarray(inputs[nm])[l][2 * D:3 * D]
            assert not np.any(vb), "nonzero v bias not supported"
    return Cfg(B, Q, T, D, H, V, L, FF, TP, n_cores, flags)


def build_inmaps(inputs, c: Cfg):
    g = lambda nm: np.asarray(inputs[nm], np.float32)
    bf = lambda a: np.ascontiguousarray(a, dtype=np.float32).astype(BF16)

    tok = np.asarray(inputs["tok_emb"], np.float32)
    posf = np.ascontiguousarray(g("pos_emb")[0, :c.T, :])
    head_w = g("head_w")
    head_t = np.concatenate([head_w[q].T for q in range(c.Q)], axis=1)

    common = {f"tok_emb_{q}": np.ascontiguousarray(tok[q])
              for q in range(c.Q)}
    common["pos"] = posf
    common["head_t"] = bf(head_t)
    if "head_b" in c.flags:
        hb = g("head_b").reshape(-1)
        common["head_b"] = np.broadcast_to(hb, (P, c.Q * c.V)).copy()

    per_tp = []
    for tp in range(c.TP):
        d = {}
        sl_d = slice(tp * c.DL, (tp + 1) * c.DL)
        sl_f = slice(tp * c.FFL, (tp + 1) * c.FFL)
        for pre, wq, wo, w1, w2 in [
                ("e", "e_qkv_w", "e_out_w", "e_ff1_w", "e_ff2_w"),
                ("d_s", "d_sqkv_w", "d_sout_w", None, None),
                ("d_c", "d_cqkv_w", "d_cout_w", None, None)]:
            qkv = g(wq)
            out_w = g(wo)
            for l in range(c.L):
                wqkv = np.concatenate(
                    [qkv[l][0:c.D][sl_d], qkv[l][c.D:2 * c.D][sl_d],
                     qkv[l][2 * c.D:3 * c.D][sl_d]], axis=0)
                nm = {"e": "e_qkv", "d_s": "d_sqkv", "d_c": "d_cqkv"}[pre]
                d[f"{nm}_{l}"] = bf(wqkv.T)
                onm = {"e": "e_out", "d_s": "d_sout", "d_c": "d_cout"}[pre]
                d[f"{onm}_{l}"] = bf(out_w[l][:, sl_d].T)
        for l in range(c.L):
            d[f"e_ff1_{l}"] = bf(g("e_ff1_w")[l][sl_f].T)
            d[f"e_ff2_{l}"] = bf(g("e_ff2_w")[l][:, sl_f].T)
            d[f"d_ff1_{l}"] = bf(g("d_ff1_w")[l][sl_f].T)
            d[f"d_ff2_{l}"] = bf(g("d_ff2_w")[l][:, sl_f].T)
        # optional biases
        for l in range(c.L):
            for knm, ref in [("e_qkv_b", "e_qkv_b"), ("d_sqkv_b", "d_sqkv_b"),
                             ("d_cqkv_b", "d_cqkv_b")]:
                if f"{knm}_{l}" in c.flags:
                    b = g(ref)[l]
                    d[f"{knm}_{l}"] = np.concatenate(
                        [b[0:c.D][sl_d], b[c.D:2 * c.D][sl_d],
                         np.zeros(c.DL, np.float32)])
            for knm in ["e_ff1_b", "d_ff1_b"]:
                if f"{knm}_{l}" in c.flags:
                    d[f"{knm}_{l}"] = np.ascontiguousarray(g(knm + "")[l][sl_f])
            for knm in ["e_out_b", "e_ff2_b", "d_sout_b", "d_cout_b",
                        "d_ff2_b"]:
                if f"{knm}_{l}" in c.flags:
                    d[f"{knm}_{l}"] = np.broadcast_to(
                        g(knm)[l], (P, c.D)).copy()
            for ln in ["e_ln1", "e_ln2", "d_ln1", "d_ln2", "d_ln3"]:
                for sfx in ["w", "b"]:
                    if f"{ln}_{sfx}_{l}" in c.flags:
                        d[f"{ln}_{sfx}_{l}"] = np.broadcast_to(
                            g(f"{ln}_{sfx}")[l], (P, c.D)).copy()
        per_tp.append(d)

    codes_in = np.asarray(inputs["input_codes"], np.int32)
    codes_tgt = np.asarray(inputs["target_codes"], np.int32)
    in_maps = []
    for core in range(c.n_cores):
        b, tp = core // c.TP, core % c.TP
        m = dict(common)
        m.update(per_tp[tp])
        m["codes_in"] = np.ascontiguousarray(codes_in[b % c.B])
        m["codes_tgt"] = np.ascontiguousarray(codes_tgt[b % c.B])
        in_maps.append(m)
    return in_maps


def postprocess(results, c: Cfg):
    out = np.empty((c.B, c.T, c.Q, c.V), np.float32)
    for b in range(c.B):
        r = results[b * c.TP]["logits"]      # [Q, T, V]
        out[b] = r.transpose(1, 0, 2)
    return out


def run(inputs, trace=False):
    from concourse.bass_utils import run_bass_kernel_spmd
    c = parse_cfg(inputs)
    key = c.key()
    if key not in _PROG_CACHE:
        _PROG_CACHE[key] = build_program(c)
    nc = _PROG_CACHE[key]
    in_maps = build_inmaps(inputs, c)
    res = run_bass_kernel_spmd(nc, in_maps, list(range(c.n_cores)),
                               trace=trace)
    return postprocess(res.results, c), res


def kernel(**inputs):
    out, _ = run(inputs, trace=False)
    return out



# revision 29
# speedup vs baseline: 1.2402x; 1.2402x over previous
"""Trainium2 Bass kernel for an encoder-decoder (S2S) transformer.

Distribution: 8 NeuronCores = 4 data-parallel groups (batch B=4) x 2-way
SEQUENCE-parallel within each pair.  Each core owns 256 tokens (2 tiles of
128) of one batch element at full model width, so layernorm, FFN and every
projection is communication-free.  Only attention needs the peer's keys and
values: one AllGather of the packed (K^T, V_aug) block per attention, issued
right after the k/v projections and consumed after the q projection and the
core's own-key score blocks, so the collective hides under compute.  The
decoder's cross-attention K/V depend only on encoder memory and are
prefetched one layer ahead (ping-pong buffers).

A single SPMD program runs on all 8 cores.  Per-core differences are data:
  - token slices of codes / positional embeddings,
  - an index tile that makes the AllGather receive pick the PEER's rows,
  - exp-bias "gates" (0 or -1e9) that implement causal masking of the peer
    key blocks; keys live in a per-core permuted order (own tokens first)
    which attention is invariant to as long as masks agree.

Layout conventions (per core, P=128, TL=256 local tokens):
  x_nat[t]  [P, D]    fp32  natural tokens-on-partitions residual stream
  x_T       [P, ND*TL] bf16 transposed: chunk d cols [d*TL:(d+1)*TL]
  q_T       [P, ND*TL] bf16 rows = head dims (2 heads/chunk, 64 rows each)
  k_T       [P, ND*T]  bf16 chunk m cols: [own 256 | peer 256]
  v_aug     [P, 4*WA]  bf16 4 key slots x (per head 64 value + 64 ones cols);
                            slots 0,1 = own tiles, 2,3 = peer tiles
  attn_T    [P, ND*TL] bf16 context, transposed
  h_T       [P, NFF*TL] bf16 ffn hidden, transposed
"""

import os
import sys

for _p in ("/opt/trn_rl_repo", "/root/.axon_site/_ro/trn_rl_repo"):
    if os.path.isdir(_p) and _p not in sys.path:
        sys.path.insert(0, _p)

import numpy as np
import ml_dtypes

import concourse.bass as bass
import concourse.bacc as bacc
import concourse.tile as tile
from concourse import mybir
from concourse.bass import IndirectOffsetOnAxis
from concourse.masks import make_identity

BF16 = ml_dtypes.bfloat16
FP8NP = ml_dtypes.float8_e4m3fn
F32 = mybir.dt.float32
BF = mybir.dt.bfloat16
F8 = mybir.dt.float8e4
I32 = mybir.dt.int32
DR = mybir.MatmulPerfMode.DoubleRow
AF = mybir.ActivationFunctionType
ALU = mybir.AluOpType
AX = mybir.AxisListType

P = 128


class Cfg:
    def __init__(self, B, Q, T, D, H, V, L, FF, n_cores, flags=frozenset(),
                 fp8=False):
        self.B, self.Q, self.T, self.D, self.H, self.V, self.L, self.FF = \
            B, Q, T, D, H, V, L, FF
        self.n_cores = n_cores
        self.FP8 = fp8
        self.WSCALE = 64.0 if fp8 else 1.0   # host-side weight prescale
        self.SP = 2
        self.E = D // Q
        self.HD = D // H
        assert self.HD == 64, "head packing assumes head_dim 64"
        assert self.E == P, "per-quantizer embedding dim must be 128"
        self.SCALE = 1.0 / float(np.sqrt(self.HD))
        self.TL = T // self.SP          # local tokens
        self.NTL = self.TL // P         # local token tiles (2)
        self.NT = T // P                # all token tiles (4)
        self.ND = D // P                # 8
        self.NFF = FF // P              # 32
        self.WA = self.H * P            # v block width per key slot (2048)
        self.KSLOT = self.ND * P        # k block width per key slot (1024)
        self.AGW = self.NTL * (self.KSLOT + self.WA)  # half width (6144)
        self.flags = frozenset(flags)

    def kcol(self, s, m=0):
        """col of chunk m of key slot s in the combined kv tile."""
        return (s // 2) * self.AGW + (s % 2) * self.KSLOT + m * P

    def vcol(self, s):
        """col of the v_aug block of key slot s in the combined kv tile."""
        return (s // 2) * self.AGW + self.NTL * self.KSLOT + (s % 2) * self.WA

    def key(self):
        return (self.B, self.Q, self.T, self.D, self.H, self.V, self.L,
                self.FF, self.n_cores, self.FP8, tuple(sorted(self.flags)))


# --------------------------------------------------------------------------
# program builder
# --------------------------------------------------------------------------

def build_program(c: Cfg):
    nc = bacc.Bacc(None, target_bir_lowering=False, num_devices=c.n_cores)

    WDT = F8 if c.FP8 else BF

    def din(name, shape, dt=None):
        if dt is None:
            dt = WDT
        return nc.dram_tensor(name, shape, dt, kind="ExternalInput")

    x_emb_in = din("x_emb", [c.TL, c.D], F32)   # host-side embed + pos
    y_emb_in = din("y_emb", [c.TL, c.D], F32)
    rridx = din("rridx", [P, 1], I32)        # peer row indices in AG output
    gates = din("gates", [P, 4], F32)        # causal slot gates (0 / -1e9)

    w = {}
    NQKV = 3 * c.ND                          # 24 qkv chunks (q 0-7, k 8-15, v 16-23)
    for l in range(c.L):
        for nm, sh in [
                (f"e_qkv_{l}", [NQKV, P, c.D]), (f"e_out_{l}", [c.ND, P, c.D]),
                (f"e_ff1_{l}", [c.NFF, P, c.D]), (f"e_ff2_{l}", [c.NFF, P, c.D]),
                (f"d_sqkv_{l}", [NQKV, P, c.D]), (f"d_sout_{l}", [c.ND, P, c.D]),
                (f"d_cqkv_{l}", [NQKV, P, c.D]), (f"d_cout_{l}", [c.ND, P, c.D]),
                (f"d_ff1_{l}", [c.NFF, P, c.D]), (f"d_ff2_{l}", [c.NFF, P, c.D])]:
            w[nm] = din(nm, sh)
    w["head_t"] = din("head_t", [c.Q, P, c.V])

    opt = {}
    for nm in c.flags:
        if "_qkv_b_" in nm or "_sqkv_b_" in nm or "_cqkv_b_" in nm:
            opt[nm] = din(nm, [P, 2 * c.ND], F32)   # q,k bias cols per chunk
        elif "_ff1_b_" in nm:
            opt[nm] = din(nm, [P, c.NFF], F32)
        elif nm == "head_b":
            opt[nm] = din(nm, [P, c.Q * c.V], F32)
        else:
            opt[nm] = din(nm, [P, c.D], F32)

    logits = nc.dram_tensor("logits", [c.Q, c.TL, c.V], F32,
                            kind="ExternalOutput")

    groups = [[g * c.SP + i for i in range(c.SP)]
              for g in range(c.n_cores // c.SP)]

    with tile.TileContext(nc) as tc:
        _emit(nc, tc, c, x_emb_in, y_emb_in, rridx, gates,
              w, opt, logits, groups, WDT)
    nc.compile()
    return nc


def _emit(nc, tc, c, x_emb_in, y_emb_in, rridx, gates,
          w, opt, logits, groups, WDT):
    from contextlib import ExitStack
    es = ExitStack()
    pool = lambda name, bufs, space="SBUF": es.enter_context(
        tc.tile_pool(name=name, bufs=bufs, space=space))

    const = pool("const", 1)
    persist = pool("persist", 1)
    wpool = pool("wpool", 4)       # lhsT-style weight chunks (qkv/ff1)
    wk = pool("wk", 6)             # rhs-style weight chunks (v/out/ff2/head)
    act = pool("act", 3)
    scratch = pool("scratch", 2)
    dram = pool("dram", 4, space="DRAM")
    ps_big = pool("ps_big", 4, space="PSUM")    # 4 banks: accum for out/ff2/v
    ps_chunk = pool("ps_chunk", 2, space="PSUM")  # 2 banks: qkv/ff1/scores/tr
    ps_att = pool("ps_att", 2, space="PSUM")    # 2 banks: AV accum

    def wdma(i, out, in_, noscalar=False):
        """Weight-stream DMA spread over queues.  noscalar keeps the
        scalar engine free (it is the bottleneck during attention)."""
        engs = [nc.sync, nc.gpsimd] if noscalar else [nc.sync, nc.scalar]
        engs[i % 2].dma_start(out=out, in_=in_)

    ISC = 1.0 / 64.0 if c.FP8 else None   # inverse weight prescale

    def evac(dsl, psl, eng, bias=None):
        """PSUM -> SBUF copy with optional 1/WSCALE and bias fold."""
        if ISC is None:
            if bias is not None:
                nc.vector.tensor_scalar(out=dsl, in0=psl, scalar1=bias,
                                        scalar2=None, op0=ALU.add)
            elif eng == "s":
                nc.scalar.copy(out=dsl, in_=psl)
            else:
                nc.vector.tensor_copy(out=dsl, in_=psl)
        else:
            if bias is not None:
                nc.vector.tensor_scalar(out=dsl, in0=psl, scalar1=ISC,
                                        scalar2=bias, op0=ALU.mult,
                                        op1=ALU.add)
            elif eng == "s":
                nc.scalar.activation(dsl, psl, AF.Copy, scale=ISC)
            else:
                nc.vector.tensor_scalar_mul(dsl, psl, ISC)

    def big_tile():
        return ps_big.tile([P, 512], F32, tag="ps_big", name="psb")

    def chunk_tile():
        t = ps_chunk.tile([P, 512], F32, tag="ps_chunk", name="psc")
        return t

    def att_tile():
        return ps_att.tile([P, 512], F32, tag="ps_att", name="psa")

    # constants
    ident_f = const.tile([P, P], F32, name="ident_f")
    make_identity(nc, ident_f[:])
    causT = const.tile([P, P], F32, name="causT")
    nc.gpsimd.memset(causT[:], 0.0)
    # transposed causal: fill -1e9 where tk > tq  (iota = tq - tk >= 0 keeps)
    nc.gpsimd.affine_select(out=causT[:], in_=causT[:],
                            compare_op=ALU.is_ge, fill=-1e9, base=0,
                            pattern=[[1, P]], channel_multiplier=-1)
    eps_t = const.tile([P, 1], F32, name="eps_t")
    nc.vector.memset(eps_t[:], 1e-5)
    gates_sb = const.tile([P, 4], F32, name="gates_sb")
    nc.sync.dma_start(out=gates_sb[:], in_=gates[:])
    rridx_sb = const.tile([P, 1], I32, name="rridx_sb")
    nc.sync.dma_start(out=rridx_sb[:], in_=rridx[:])

    opt_sb = {}
    for nm in opt:
        shp = list(opt[nm].shape)
        t = const.tile(shp, F32, name=f"sb_{nm}")
        nc.sync.dma_start(out=t[:], in_=opt[nm][:])
        opt_sb[nm] = t

    # persistent activations
    x_nat = [persist.tile([P, c.D], F32, name=f"x_{t}", tag=f"x_{t}")
             for t in range(c.NTL)]
    y_emb = [persist.tile([P, c.D], F32, name=f"y_{t}", tag=f"y_{t}")
             for t in range(c.NTL)]
    ADT = F8 if c.FP8 else BF        # fat-matmul activation dtype
    x_T = persist.tile([P, c.ND * c.TL], ADT, name="x_T", tag="x_T")
    mem_T = persist.tile([P, c.ND * c.TL], ADT, name="mem_T", tag="mem_T")
    q_T = persist.tile([P, c.ND * c.TL], BF, name="q_T", tag="q_T")
    # combined (K^T, V_aug) tiles: [own half 6144 | peer half 6144]; each
    # half is [k slot, k slot, v slot, v slot] so the AG send/recv are single
    # contiguous row blocks.
    kv = persist.tile([P, 2 * c.AGW], BF, name="kv", tag="kv")
    ckv = [persist.tile([P, 2 * c.AGW], BF, name=f"ckv{i}", tag=f"ckv{i}")
           for i in range(2)]
    attn_T = persist.tile([P, c.ND * c.TL], ADT, name="attn_T", tag="attn_T")
    h_T = persist.tile([P, c.NFF * c.TL], ADT, name="h_T", tag="h_T")

    # ones columns of the v_aug blocks (set once; projections/recv write the
    # value cols, and the AG transports the peer's ones columns verbatim)
    for kvt in [kv] + ckv:
        for s in range(c.NT):
            blk = kvt[:, c.vcol(s):c.vcol(s) + c.WA].rearrange(
                "p (h f) -> p h f", f=P)
            nc.vector.memset(blk[:, :, 64:128], 1.0)

    # ---------------- helpers ----------------
    def transpose_tile(dst, t):
        """dst[:, d*TL + t*P : +P] = x_nat[t][:, d*P:+P].T (fp32 cast)."""
        for d in range(c.ND):
            pt = ps_chunk.tile([P, P], F32, tag="ps_chunk", name="pt")
            nc.tensor.transpose(pt[:], x_nat[t][:, d * P:(d + 1) * P],
                                ident_f[:])
            dsl = dst[:, d * c.TL + t * P: d * c.TL + t * P + P]
            if (t + d) % 2 == 0:
                nc.scalar.copy(out=dsl, in_=pt[:])
            else:
                nc.vector.tensor_copy(out=dsl, in_=pt[:])

    def transpose_x_into(dst):
        for t in range(c.NTL):
            transpose_tile(dst, t)

    def embed(src_dram, dst):
        for t in range(c.NTL):
            nc.sync.dma_start(out=dst[t][:],
                              in_=src_dram[t * P:(t + 1) * P, :])

    def ln_tile(t, sub, gname, bname):
        """x_nat[t] = LN(x_nat[t] + sub) * g + b (post-norm)."""
        xt = x_nat[t]
        s1 = scratch.tile([P, 1], F32, tag="lnstat", name="s1", bufs=8)
        s2 = scratch.tile([P, 1], F32, tag="lnstat", name="s2", bufs=8)
        sq = scratch.tile([P, c.D], BF, tag="lnsq", name="sq")
        nc.vector.tensor_tensor(out=xt[:], in0=xt[:], in1=sub[:], op=ALU.add)
        nc.vector.reduce_sum(out=s1[:], in_=xt[:], axis=AX.X)
        nc.scalar.activation(sq[:], xt[:], AF.Square, accum_out=s2[:])
        mean = scratch.tile([P, 1], F32, tag="lnstat", name="mean", bufs=8)
        var = scratch.tile([P, 1], F32, tag="lnstat", name="var", bufs=8)
        m2 = scratch.tile([P, 1], F32, tag="lnstat", name="m2", bufs=8)
        nc.vector.tensor_scalar_mul(mean[:], s1[:], 1.0 / c.D)
        nc.vector.tensor_scalar_mul(var[:], s2[:], 1.0 / c.D)
        nc.vector.tensor_tensor(out=m2[:], in0=mean[:], in1=mean[:],
                                op=ALU.mult)
        nc.vector.tensor_tensor(out=var[:], in0=var[:], in1=m2[:],
                                op=ALU.subtract)
        rstd = scratch.tile([P, 1], F32, tag="lnstat", name="rstd", bufs=8)
        nc.scalar.activation(rstd[:], var[:], AF.Sqrt, bias=eps_t[:])
        nc.vector.reciprocal(rstd[:], rstd[:])
        nc.vector.tensor_scalar(out=xt[:], in0=xt[:], scalar1=mean[:],
                                scalar2=rstd[:], op0=ALU.subtract,
                                op1=ALU.mult)
        if gname in opt_sb:
            nc.vector.tensor_tensor(out=xt[:], in0=xt[:],
                                    in1=opt_sb[gname][:], op=ALU.mult)
        if bname in opt_sb:
            nc.vector.tensor_tensor(out=xt[:], in0=xt[:],
                                    in1=opt_sb[bname][:], op=ALU.add)

    def proj_qk(wname, bname, src_T, dst, kind):
        """kind='q': chunks 0-7 -> dst[:, m*TL cols]; kind='k': chunks 8-15
        -> the own key slots of the combined kv tile `dst`."""
        cbase = 0 if kind == "q" else c.ND
        for m in range(c.ND):
            wt = wpool.tile([P, c.D], WDT, tag="wqkv", name="wt")
            wdma(m, wt[:], w[wname][cbase + m], noscalar=True)
            ps = chunk_tile()[:, 0:c.TL]
            if c.FP8:
                wt3 = wt[:].rearrange("p (k m) -> p k m", m=P)
                x3 = src_T[:].rearrange("p (k t) -> p k t", t=c.TL)
                for j in range(c.ND // 2):
                    nc.tensor.matmul(ps[:], wt3[:, 2 * j:2 * j + 2, :],
                                     x3[:, 2 * j:2 * j + 2, :],
                                     start=(j == 0), stop=(j == c.ND // 2 - 1),
                                     perf_mode=DR)
            else:
                for k in range(c.ND):
                    nc.tensor.matmul(ps[:], wt[:, k * P:(k + 1) * P],
                                     src_T[:, k * c.TL:(k + 1) * c.TL],
                                     start=(k == 0), stop=(k == c.ND - 1))
            if bname in opt_sb:
                bcol = m if kind == "q" else c.ND + m
                bias = opt_sb[bname][:, bcol:bcol + 1]
            else:
                bias = None
            if kind == "q":
                evac(dst[:, m * c.TL:(m + 1) * c.TL], ps[:],
                     "v" if m % 2 == 0 else "s", bias)
            else:
                for tl in range(c.NTL):
                    dsl = dst[:, c.kcol(tl, m):c.kcol(tl, m) + P]
                    psl = ps[:, tl * P:(tl + 1) * P]
                    evac(dsl, psl, "v", bias)

    def proj_v(wname, src_T, dst_aug):
        """chunks 16-23 (rhs layout): natural v for own tiles -> dst_aug
        slots 0..NTL-1 data columns."""
        NB = c.D // 512
        pss = [[big_tile() for n in range(NB)] for t in range(c.NTL)]
        if c.FP8:
            src3 = src_T[:].rearrange("p (k t) -> p k t", t=c.TL)
            for j in range(c.ND // 2):
                wt = wk.tile([P, 2 * c.D], F8, tag="wv", name="wt")
                wt3 = wt[:].rearrange("p (k d) -> p k d", d=c.D)
                wdma(j, wt3,
                     w[wname][2 * c.ND + 2 * j:2 * c.ND + 2 * j + 2]
                     .rearrange("k p d -> p k d"))
                for t in range(c.NTL):
                    for n in range(NB):
                        nc.tensor.matmul(
                            pss[t][n][:],
                            src3[:, 2 * j:2 * j + 2, t * P:t * P + P],
                            wt3[:, :, n * 512:(n + 1) * 512],
                            start=(j == 0), stop=(j == c.ND // 2 - 1),
                            perf_mode=DR)
        else:
            for k in range(c.ND):
                wt = wk.tile([P, c.D], BF, tag="wv", name="wt")
                wdma(k, wt[:], w[wname][2 * c.ND + k], noscalar=True)
                for t in range(c.NTL):
                    for n in range(NB):
                        nc.tensor.matmul(
                            pss[t][n][:],
                            src_T[:, k * c.TL + t * P: k * c.TL + t * P + P],
                            wt[:, n * 512:(n + 1) * 512],
                            start=(k == 0), stop=(k == c.ND - 1))
        for t in range(c.NTL):
            blk = dst_aug[:, c.vcol(t):c.vcol(t) + c.WA].rearrange(
                "p (h f) -> p h f", f=P)
            for n in range(NB):
                # 512 cols = 8 heads' worth of 64-wide value blocks
                psv = pss[t][n][:].rearrange("p (h f) -> p h f", f=64)
                dstb = blk[:, n * 8:(n + 1) * 8, 0:64]
                if c.FP8:
                    nc.scalar.activation(dstb, psv, AF.Copy, scale=1.0 / 64.0)
                else:
                    nc.scalar.copy(out=dstb, in_=psv)

    KR = c.NTL * c.KSLOT            # k region width per half (2048)
    VR = c.AGW - KR

    def ag_start(kv_dst, lo, hi, tag):
        """Send + AllGather-trigger for kv_dst[:, lo:hi].  The receive is a
        SEPARATE call so triggers never queue behind earlier receives on the
        in-order gpsimd stream."""
        agin = dram.tile([P, hi - lo], BF, tag=f"agin{tag}", name="agin")
        agout = dram.tile([c.SP * P, hi - lo], BF, tag=f"agout{tag}",
                          name="agout")
        nc.sync.dma_start(out=agin[:], in_=kv_dst[:, lo:hi])
        nc.gpsimd.collective_compute(
            "AllGather", ALU.bypass, replica_groups=groups,
            ins=[agin[:].opt()], outs=[agout[:].opt()])
        return (agout, kv_dst, c.AGW + lo)

    def ag_recv(h):
        agout, kv_dst, dst0 = h
        n = agout.shape[-1]
        nc.gpsimd.indirect_dma_start(
            out=kv_dst[:, dst0:dst0 + n],
            out_offset=None,
            in_=agout[:],
            in_offset=IndirectOffsetOnAxis(ap=rridx_sb[:, :1], axis=0))

    def ag_k_start(kv_dst):
        return ag_start(kv_dst, 0, KR, "k")

    def ag_v_start(kv_dst):
        return ag_start(kv_dst, KR, c.AGW, "v")

    def attention(kvt, mode):
        """q_T x kvt -> attn_T.  mode: 'full' (all 4 slots, no mask) or
        'causal' (slot structure for decoder self-attention)."""
        for h in range(c.H):
            m = h // 2
            po = 64 * (h % 2)
            # score slot pairs share one PSUM bank + one exp where possible:
            # pair block cols [s%2 * TL : +TL]; in causal mode slot 1 only
            # has q cols [P:TL] (placed at [TL+P:2TL]) and slots 0/1 get the
            # diagonal causT mask; slots 2/3 share one gate bias.
            at = []
            for pi, (s0, s1) in enumerate([(0, 1), (2, 3)]):
                pss = big_tile()
                a = act.tile([P, 2 * c.TL], BF, tag=f"ATp{pi}", name="at",
                             bufs=3)
                qr = []
                for si, s in enumerate((s0, s1)):
                    q0 = P if (mode == "causal" and s == 1) else 0
                    base = si * c.TL
                    nc.tensor.matmul(
                        pss[:, base + q0: base + c.TL],
                        kvt[po:po + 64, c.kcol(s, m):c.kcol(s, m) + P],
                        q_T[po:po + 64, m * c.TL + q0: m * c.TL + c.TL],
                        start=True, stop=True)
                    if mode == "causal" and s < 2:
                        # diagonal block: q columns s*P of this slot
                        d0 = base + s * P
                        nc.vector.tensor_tensor(
                            out=pss[:, d0:d0 + P], in0=pss[:, d0:d0 + P],
                            in1=causT[:], op=ALU.add)
                    qr.append(q0)
                if mode == "causal" and pi == 1:
                    nc.scalar.activation(a[:], pss[:], AF.Exp, scale=c.SCALE,
                                         bias=gates_sb[:, s0:s0 + 1])
                elif mode == "causal":
                    # two exps: slot0 full, slot1 partial (skip the dead gap)
                    nc.scalar.activation(a[:, 0:c.TL], pss[:, 0:c.TL],
                                         AF.Exp, scale=c.SCALE)
                    nc.scalar.activation(a[:, c.TL + P:2 * c.TL],
                                         pss[:, c.TL + P:2 * c.TL],
                                         AF.Exp, scale=c.SCALE)
                else:
                    nc.scalar.activation(a[:], pss[:], AF.Exp, scale=c.SCALE)
                at.append((s0, a, 0, qr[0]))
                at.append((s1, a, c.TL, qr[1]))
            ps_o = att_tile()[:, 0:c.TL]
            for i, (s, a, base, q0) in enumerate(at):
                nc.tensor.matmul(
                    ps_o[:, q0:c.TL],
                    kvt[:, c.vcol(s) + h * P: c.vcol(s) + (h + 1) * P],
                    a[:, base + q0: base + c.TL],
                    start=(i == 0), stop=(i == len(at) - 1))
            rden = scratch.tile([64, c.TL], F32, tag="rden", name="rden",
                                bufs=2)
            nc.vector.reciprocal(rden[:], ps_o[64:128, :])
            nc.vector.tensor_tensor(
                out=attn_T[po:po + 64, m * c.TL:(m + 1) * c.TL],
                in0=ps_o[0:64, :], in1=rden[:], op=ALU.mult)

    def mm_to_natural(src_T, nk, wname, bname, noscalar=False):
        """[TL, D] = src_T.T @ W (k-chunk streaming, PSUM accumulate),
        returned as per-t bf16 [P, D] tiles."""
        NB = c.D // 512
        pss = [[big_tile() for n in range(NB)] for t in range(c.NTL)]
        if c.FP8:
            src3 = src_T[:].rearrange("p (k t) -> p k t", t=c.TL)
            for j in range(nk // 2):
                wt = wk.tile([P, 2 * c.D], F8, tag="wnat", name="wt")
                wt3 = wt[:].rearrange("p (k d) -> p k d", d=c.D)
                wdma(j, wt3,
                     w[wname][2 * j:2 * j + 2].rearrange("k p d -> p k d"))
                for t in range(c.NTL):
                    for n in range(NB):
                        nc.tensor.matmul(
                            pss[t][n][:],
                            src3[:, 2 * j:2 * j + 2, t * P:t * P + P],
                            wt3[:, :, n * 512:(n + 1) * 512],
                            start=(j == 0), stop=(j == nk // 2 - 1),
                            perf_mode=DR)
        else:
            for k in range(nk):
                wt = wk.tile([P, c.D], BF, tag="wnat", name="wt")
                wdma(k, wt[:], w[wname][k], noscalar=noscalar)
                for t in range(c.NTL):
                    for n in range(NB):
                        nc.tensor.matmul(
                            pss[t][n][:],
                            src_T[:, k * c.TL + t * P: k * c.TL + t * P + P],
                            wt[:, n * 512:(n + 1) * 512],
                            start=(k == 0), stop=(k == nk - 1))
        parts = []
        for t in range(c.NTL):
            sb = scratch.tile([P, c.D], BF, tag="oproj", name="sb", bufs=3)
            for n in range(NB):
                evac(sb[:, n * 512:(n + 1) * 512], pss[t][n][:],
                     "s" if (t + n) % 2 == 0 else "v")
            if bname in opt_sb:
                nc.vector.tensor_tensor(out=sb[:], in0=sb[:],
                                        in1=opt_sb[bname][:], op=ALU.add)
            parts.append(sb)
        return parts

    def ffn(w1name, b1name, w2name, b2name):
        x3 = x_T[:].rearrange("p (k t) -> p k t", t=c.TL)
        for mchunk in range(c.NFF):
            wt = wpool.tile([P, c.D], WDT, tag="wff1", name="wt")
            wdma(mchunk, wt[:], w[w1name][mchunk])
            ps = chunk_tile()[:, 0:c.TL]
            if c.FP8:
                wt3 = wt[:].rearrange("p (k m) -> p k m", m=P)
                for j in range(c.ND // 2):
                    nc.tensor.matmul(ps[:], wt3[:, 2 * j:2 * j + 2, :],
                                     x3[:, 2 * j:2 * j + 2, :],
                                     start=(j == 0), stop=(j == c.ND // 2 - 1),
                                     perf_mode=DR)
            else:
                for k in range(c.ND):
                    nc.tensor.matmul(ps[:], wt[:, k * P:(k + 1) * P],
                                     x_T[:, k * c.TL:(k + 1) * c.TL],
                                     start=(k == 0), stop=(k == c.ND - 1))
            dsl = h_T[:, mchunk * c.TL:(mchunk + 1) * c.TL]
            kw = {"scale": 1.0 / 64.0} if c.FP8 else {}
            if b1name in opt_sb:
                nc.scalar.activation(dsl, ps[:], AF.Relu,
                                     bias=opt_sb[b1name][:, mchunk:mchunk + 1],
                                     **kw)
            else:
                nc.scalar.activation(dsl, ps[:], AF.Relu, **kw)
        return mm_to_natural(h_T, c.NFF, w2name, b2name)

    def ag_warm(dep_tile):
        """Tiny AllGather that wakes the cc stream ahead of a real AG;
        dep_tile pins its position in the schedule."""
        n = dep_tile.shape[-1]
        win = dram.tile([P, n], dep_tile.dtype, tag="warmin", name="win")
        wout = dram.tile([c.SP * P, n], dep_tile.dtype, tag="warmout",
                         name="wout")
        nc.sync.dma_start(out=win[:], in_=dep_tile)
        nc.gpsimd.collective_compute(
            "AllGather", ALU.bypass, replica_groups=groups,
            ins=[win[:].opt()], outs=[wout[:].opt()])

    def cross_kv_proj(l):
        buf = l % 2
        proj_qk(f"d_cqkv_{l}", f"d_cqkv_b_{l}", mem_T, ckv[buf], "k")
        proj_v(f"d_cqkv_{l}", mem_T, ckv[buf])

    def cross_kv_prefetch(l):
        """Project + AllGather cross-attention K/V for decoder layer l."""
        cross_kv_proj(l)
        hk = ag_k_start(ckv[l % 2])
        hv = ag_v_start(ckv[l % 2])
        ag_recv(hk)
        ag_recv(hv)

    # ---------------- encoder ----------------
    ag_warm(gates_sb[:, 0:4])
    embed(x_emb_in, x_nat)
    embed(y_emb_in, y_emb)
    transpose_x_into(x_T)
    for l in range(c.L):
        proj_qk(f"e_qkv_{l}", f"e_qkv_b_{l}", x_T, kv, "k")
        if l > 0:
            ag_warm(kv[:, 0:8])
        hk = ag_k_start(kv)
        proj_v(f"e_qkv_{l}", x_T, kv)
        hv = ag_v_start(kv)
        ag_recv(hk)
        ag_recv(hv)
        proj_qk(f"e_qkv_{l}", f"e_qkv_b_{l}", x_T, q_T, "q")
        attention(kv, "full")
        parts = mm_to_natural(attn_T, c.ND, f"e_out_{l}", f"e_out_b_{l}",
                              noscalar=True)
        for t in range(c.NTL):
            ln_tile(t, parts[t], f"e_ln1_w_{l}", f"e_ln1_b_{l}")
            transpose_tile(x_T, t)
        parts = ffn(f"e_ff1_{l}", f"e_ff1_b_{l}", f"e_ff2_{l}", f"e_ff2_b_{l}")
        last = l == c.L - 1
        for t in range(c.NTL):
            ln_tile(t, parts[t], f"e_ln2_w_{l}", f"e_ln2_b_{l}")
            transpose_tile(mem_T if last else x_T, t)

    # cross K/V for decoder layer 0 (hides under decoder embed + self attn)
    cross_kv_prefetch(0)

    # ---------------- decoder ----------------
    for t in range(c.NTL):
        nc.vector.tensor_copy(out=x_nat[t][:], in_=y_emb[t][:])
        transpose_tile(x_T, t)
    for l in range(c.L):
        proj_qk(f"d_sqkv_{l}", f"d_sqkv_b_{l}", x_T, kv, "k")
        ag_warm(kv[:, 0:8])
        hk = ag_k_start(kv)
        proj_v(f"d_sqkv_{l}", x_T, kv)
        hv = ag_v_start(kv)
        # cross K/V for the NEXT layer: the projections are PE cover work for
        # the self AG flight; their triggers ride the warm cc stream and the
        # receives come after the self receives
        hck = hcv = None
        if l + 1 < c.L:
            cross_kv_proj(l + 1)
            hck = ag_k_start(ckv[(l + 1) % 2])
            hcv = ag_v_start(ckv[(l + 1) % 2])
        ag_recv(hk)
        ag_recv(hv)
        if hck is not None:
            ag_recv(hck)
            ag_recv(hcv)
        proj_qk(f"d_sqkv_{l}", f"d_sqkv_b_{l}", x_T, q_T, "q")
        attention(kv, "causal")
        parts = mm_to_natural(attn_T, c.ND, f"d_sout_{l}", f"d_sout_b_{l}",
                              noscalar=True)
        for t in range(c.NTL):
            ln_tile(t, parts[t], f"d_ln1_w_{l}", f"d_ln1_b_{l}")
            transpose_tile(x_T, t)
        proj_qk(f"d_cqkv_{l}", f"d_cqkv_b_{l}", x_T, q_T, "q")
        attention(ckv[l % 2], "full")
        parts = mm_to_natural(attn_T, c.ND, f"d_cout_{l}", f"d_cout_b_{l}",
                              noscalar=True)
        for t in range(c.NTL):
            ln_tile(t, parts[t], f"d_ln2_w_{l}", f"d_ln2_b_{l}")
            transpose_tile(x_T, t)
        parts = ffn(f"d_ff1_{l}", f"d_ff1_b_{l}", f"d_ff2_{l}", f"d_ff2_b_{l}")
        for t in range(c.NTL):
            ln_tile(t, parts[t], f"d_ln3_w_{l}", f"d_ln3_b_{l}")
            transpose_tile(x_T, t)

    # ---------------- output head ----------------
    NBV = c.V // 512
    for j in range(c.Q):
        hw = wk.tile([P, c.V], WDT, tag="whead", name="hw", bufs=2)
        wdma(j, hw[:], w["head_t"][j])
        for t in range(c.NTL):
            sb = scratch.tile([P, c.V], F32, tag="lgt", name="sb", bufs=3)
            for n in range(NBV):
                ps = big_tile()
                nc.tensor.matmul(
                    ps[:], x_T[:, j * c.TL + t * P: j * c.TL + t * P + P],
                    hw[:, n * 512:(n + 1) * 512], start=True, stop=True)
                evac(sb[:, n * 512:(n + 1) * 512], ps[:], "s")
            if "head_b" in opt_sb:
                nc.vector.tensor_tensor(
                    out=sb[:], in0=sb[:],
                    in1=opt_sb["head_b"][:, j * c.V:(j + 1) * c.V],
                    op=ALU.add)
            nc.sync.dma_start(out=logits[j, t * P:(t + 1) * P, :], in_=sb[:])

    es.close()


# --------------------------------------------------------------------------
# host side
# --------------------------------------------------------------------------

_PROG_CACHE = {}


def parse_cfg(inputs, n_cores=8, fp8=None):
    B, Q, T = inputs["input_codes"].shape
    _, V, E = np.asarray(inputs["tok_emb"]).shape
    L, _, D = np.asarray(inputs["e_qkv_w"]).shape
    FF = np.asarray(inputs["e_ff1_w"]).shape[1]
    H = D // 64
    flags = set()
    for l in range(L):
        for knm in ["e_qkv_b", "d_sqkv_b", "d_cqkv_b", "e_ff1_b", "d_ff1_b",
                    "e_out_b", "e_ff2_b", "d_sout_b", "d_cout_b", "d_ff2_b"]:
            if np.any(np.asarray(inputs[knm])[l]):
                flags.add(f"{knm}_{l}")
        for ln in ["e_ln1", "e_ln2", "d_ln1", "d_ln2", "d_ln3"]:
            if not np.all(np.asarray(inputs[ln + "_w"])[l] == 1.0):
                flags.add(f"{ln}_w_{l}")
            if np.any(np.asarray(inputs[ln + "_b"])[l]):
                flags.add(f"{ln}_b_{l}")
    if np.any(np.asarray(inputs["head_b"])):
        flags.add("head_b")
    if fp8 is None:
        fp8 = os.environ.get("BASS_S2S_FP8", "0") == "1"
    # v-bias unsupported in-kernel; fall back assertion
    for l in range(L):
        for nm in ["e_qkv_b", "d_sqkv_b", "d_cqkv_b"]:
            vb = np.asarray(inputs[nm])[l][2 * D:3 * D]
            assert not np.any(vb), "nonzero v bias not supported"
    return Cfg(B, Q, T, D, H, V, L, FF, n_cores, flags, fp8=fp8)


def _lhsT_chunks(wm, D):
    """[M, D] row-major weight -> [M//128, 128(p=in%128), ...] lhsT chunk
    layout: chunk c element [p, nd*128 + m] = wm[c*128 + m, nd*128 + p]."""
    M = wm.shape[0]
    nd = D // P
    out = np.empty((M // P, P, D), np.float32)
    for cc in range(M // P):
        wc = wm[cc * P:(cc + 1) * P, :]          # [128 m, D in]
        out[cc] = wc.T.reshape(nd, P, P).transpose(1, 0, 2).reshape(P, D)
    return out


def _rhs_chunks(wm, D_out):
    """[D_out, K] row-major weight -> [K//128, 128(p=k%128), D_out] rhs
    chunk layout: chunk k element [p, n] = wm[n, k*128 + p]."""
    K = wm.shape[1]
    return np.ascontiguousarray(
        wm.T.reshape(K // P, P, D_out))


def build_inmaps(inputs, c: Cfg):
    g = lambda nm: np.asarray(inputs[nm], np.float32)
    if c.FP8:
        def bf(a):
            a = np.ascontiguousarray(a, dtype=np.float32) * c.WSCALE
            return np.clip(a, -240.0, 240.0).astype(FP8NP)
    else:
        bf = lambda a: np.ascontiguousarray(a, dtype=np.float32).astype(BF16)

    tok = np.asarray(inputs["tok_emb"], np.float32)
    posf = np.ascontiguousarray(g("pos_emb")[0, :c.T, :])
    head_w = g("head_w")

    common = {}
    head_t = np.stack([head_w[q].T for q in range(c.Q)])    # [Q, E, V]
    common["head_t"] = bf(head_t)
    if "head_b" in c.flags:
        hb = g("head_b").reshape(-1)
        common["head_b"] = np.broadcast_to(hb, (P, c.Q * c.V)).copy()

    for pre, wq, wo in [("e_qkv", "e_qkv_w", None), ("e_out", None, "e_out_w"),
                        ("d_sqkv", "d_sqkv_w", None),
                        ("d_sout", None, "d_sout_w"),
                        ("d_cqkv", "d_cqkv_w", None),
                        ("d_cout", None, "d_cout_w")]:
        for l in range(c.L):
            if wq is not None:
                qkv = g(wq)[l]                    # [3D, D]
                qk = _lhsT_chunks(qkv[0:2 * c.D], c.D)      # q,k chunks
                vv = _rhs_chunks(qkv[2 * c.D:3 * c.D], c.D)
                common[f"{pre}_{l}"] = bf(np.concatenate([qk, vv], axis=0))
            else:
                wo_l = g(wo)[l]                   # [D, D] rows = out dim
                common[f"{pre}_{l}"] = bf(_rhs_chunks(wo_l, c.D))
    for l in range(c.L):
        common[f"e_ff1_{l}"] = bf(_lhsT_chunks(g("e_ff1_w")[l], c.D))
        common[f"d_ff1_{l}"] = bf(_lhsT_chunks(g("d_ff1_w")[l], c.D))
        common[f"e_ff2_{l}"] = bf(_rhs_chunks(g("e_ff2_w")[l], c.D))
        common[f"d_ff2_{l}"] = bf(_rhs_chunks(g("d_ff2_w")[l], c.D))

    # optional biases
    for l in range(c.L):
        for knm in ["e_qkv_b", "d_sqkv_b", "d_cqkv_b"]:
            if f"{knm}_{l}" in c.flags:
                b = g(knm)[l][0:2 * c.D]           # q,k bias only
                common[f"{knm}_{l}"] = np.ascontiguousarray(
                    b.reshape(2 * c.ND, P).T)
        for knm in ["e_ff1_b", "d_ff1_b"]:
            if f"{knm}_{l}" in c.flags:
                common[f"{knm}_{l}"] = np.ascontiguousarray(
                    g(knm)[l].reshape(c.NFF, P).T)
        for knm in ["e_out_b", "e_ff2_b", "d_sout_b", "d_cout_b", "d_ff2_b"]:
            if f"{knm}_{l}" in c.flags:
                common[f"{knm}_{l}"] = np.broadcast_to(
                    g(knm)[l], (P, c.D)).copy()
        for ln in ["e_ln1", "e_ln2", "d_ln1", "d_ln2", "d_ln3"]:
            for sfx in ["w", "b"]:
                if f"{ln}_{sfx}_{l}" in c.flags:
                    common[f"{ln}_{sfx}_{l}"] = np.broadcast_to(
                        g(f"{ln}_{sfx}")[l], (P, c.D)).copy()

    codes_in = np.asarray(inputs["input_codes"], np.int32)
    codes_tgt = np.asarray(inputs["target_codes"], np.int32)

    def embed_host(codes_bqt):
        # [Q, T] codes -> [T, D] embedding (concat per-quantizer) + pos
        e = np.concatenate([tok[q][codes_bqt[q]] for q in range(c.Q)],
                           axis=-1)
        return e + posf

    emb_in = [embed_host(codes_in[b]) for b in range(c.B)]
    emb_tgt = [embed_host(codes_tgt[b]) for b in range(c.B)]
    in_maps = []
    for core in range(c.n_cores):
        b, h = core // c.SP, core % c.SP
        m = dict(common)
        sl = slice(h * c.TL, (h + 1) * c.TL)
        m["x_emb"] = np.ascontiguousarray(emb_in[b % c.B][sl])
        m["y_emb"] = np.ascontiguousarray(emb_tgt[b % c.B][sl])
        m["rridx"] = ((1 - h) * P + np.arange(P, dtype=np.int32)
                      ).reshape(P, 1)
        gate = np.zeros((P, 4), np.float32)
        if h == 0:
            gate[:, 2] = -1e9
            gate[:, 3] = -1e9
        m["gates"] = gate
        in_maps.append(m)
    return in_maps


def postprocess(results, c: Cfg):
    out = np.empty((c.B, c.T, c.Q, c.V), np.float32)
    for b in range(c.B):
        for h in range(c.SP):
            r = results[b * c.SP + h]["logits"]      # [Q, TL, V]
            out[b, h * c.TL:(h + 1) * c.TL] = r.transpose(1, 0, 2)
    return out


def run(inputs, trace=False):
    from concourse.bass_utils import run_bass_kernel_spmd
    c = parse_cfg(inputs)
    key = c.key()
    if key not in _PROG_CACHE:
        _PROG_CACHE[key] = build_program(c)
    nc = _PROG_CACHE[key]
    in_maps = build_inmaps(inputs, c)
    res = run_bass_kernel_spmd(nc, in_maps, list(range(c.n_cores)),
                               trace=trace)
    return postprocess(res.results, c), res


def kernel(**inputs):
    out, _ = run(inputs, trace=False)
    return out


# revision 31
# speedup vs baseline: 1.2969x; 1.0457x over previous
"""Trainium2 Bass kernel for an encoder-decoder (S2S) transformer.

Distribution: 8 NeuronCores = 4 data-parallel groups (batch B=4) x 2-way
SEQUENCE-parallel within each pair.  Each core owns 256 tokens (2 tiles of
128) of one batch element at full model width, so layernorm, FFN and every
projection is communication-free.  Only attention needs the peer's keys and
values: one AllGather of the packed (K^T, V_aug) block per attention, issued
right after the k/v projections and consumed after the q projection and the
core's own-key score blocks, so the collective hides under compute.  The
decoder's cross-attention K/V depend only on encoder memory and are
prefetched one layer ahead (ping-pong buffers).

A single SPMD program runs on all 8 cores.  Per-core differences are data:
  - token slices of codes / positional embeddings,
  - an index tile that makes the AllGather receive pick the PEER's rows,
  - exp-bias "gates" (0 or -1e9) that implement causal masking of the peer
    key blocks; keys live in a per-core permuted order (own tokens first)
    which attention is invariant to as long as masks agree.

Layout conventions (per core, P=128, TL=256 local tokens):
  x_nat[t]  [P, D]    fp32  natural tokens-on-partitions residual stream
  x_T       [P, ND*TL] bf16 transposed: chunk d cols [d*TL:(d+1)*TL]
  q_T       [P, ND*TL] bf16 rows = head dims (2 heads/chunk, 64 rows each)
  k_T       [P, ND*T]  bf16 chunk m cols: [own 256 | peer 256]
  v_aug     [P, 4*WA]  bf16 4 key slots x (per head 64 value + 64 ones cols);
                            slots 0,1 = own tiles, 2,3 = peer tiles
  attn_T    [P, ND*TL] bf16 context, transposed
  h_T       [P, NFF*TL] bf16 ffn hidden, transposed
"""

import os
import sys

for _p in ("/opt/trn_rl_repo", "/root/.axon_site/_ro/trn_rl_repo"):
    if os.path.isdir(_p) and _p not in sys.path:
        sys.path.insert(0, _p)

import numpy as np
import ml_dtypes

import concourse.bass as bass
import concourse.bacc as bacc
import concourse.tile as tile
from concourse import mybir
from concourse.bass import IndirectOffsetOnAxis
from concourse.masks import make_identity

BF16 = ml_dtypes.bfloat16
FP8NP = ml_dtypes.float8_e4m3fn
F32 = mybir.dt.float32
BF = mybir.dt.bfloat16
F8 = mybir.dt.float8e4
I32 = mybir.dt.int32
DR = mybir.MatmulPerfMode.DoubleRow
AF = mybir.ActivationFunctionType
ALU = mybir.AluOpType
AX = mybir.AxisListType

P = 128


class Cfg:
    def __init__(self, B, Q, T, D, H, V, L, FF, n_cores, flags=frozenset(),
                 fp8=False):
        self.B, self.Q, self.T, self.D, self.H, self.V, self.L, self.FF = \
            B, Q, T, D, H, V, L, FF
        self.n_cores = n_cores
        self.FP8 = fp8
        self.WSCALE = 64.0 if fp8 else 1.0   # host-side weight prescale
        self.SP = 2
        self.E = D // Q
        self.HD = D // H
        assert self.HD == 64, "head packing assumes head_dim 64"
        assert self.E == P, "per-quantizer embedding dim must be 128"
        self.SCALE = 1.0 / float(np.sqrt(self.HD))
        self.TL = T // self.SP          # local tokens
        self.NTL = self.TL // P         # local token tiles (2)
        self.NT = T // P                # all token tiles (4)
        self.ND = D // P                # 8
        self.NFF = FF // P              # 32
        self.WA = self.H * P            # v block width per key slot (2048)
        self.KSLOT = self.ND * P        # k block width per key slot (1024)
        self.AGW = self.NTL * (self.KSLOT + self.WA)  # half width (6144)
        self.flags = frozenset(flags)

    def kcol(self, s, m=0):
        """col of chunk m of key slot s in the combined kv tile."""
        return (s // 2) * self.AGW + (s % 2) * self.KSLOT + m * P

    def vcol(self, s):
        """col of the v_aug block of key slot s in the combined kv tile."""
        return (s // 2) * self.AGW + self.NTL * self.KSLOT + (s % 2) * self.WA

    def key(self):
        return (self.B, self.Q, self.T, self.D, self.H, self.V, self.L,
                self.FF, self.n_cores, self.FP8, tuple(sorted(self.flags)))


# --------------------------------------------------------------------------
# program builder
# --------------------------------------------------------------------------

def build_program(c: Cfg):
    nc = bacc.Bacc(None, target_bir_lowering=False, num_devices=c.n_cores)

    WDT = F8 if c.FP8 else BF

    def din(name, shape, dt=None):
        if dt is None:
            dt = WDT
        return nc.dram_tensor(name, shape, dt, kind="ExternalInput")

    x_emb_in = din("x_emb", [c.TL, c.D], F32)   # host-side embed + pos
    y_emb_in = din("y_emb", [c.TL, c.D], F32)
    rridx = din("rridx", [P, 1], I32)        # peer row indices in AG output
    gates = din("gates", [P, 4], F32)        # causal slot gates (0 / -1e9)

    w = {}
    NQKV = 3 * c.ND                          # 24 qkv chunks (q 0-7, k 8-15, v 16-23)
    for l in range(c.L):
        for nm, sh in [
                (f"e_qkv_{l}", [NQKV, P, c.D]), (f"e_out_{l}", [c.ND, P, c.D]),
                (f"e_ff1_{l}", [c.NFF, P, c.D]), (f"e_ff2_{l}", [c.NFF, P, c.D]),
                (f"d_sqkv_{l}", [NQKV, P, c.D]), (f"d_sout_{l}", [c.ND, P, c.D]),
                (f"d_cqkv_{l}", [NQKV, P, c.D]), (f"d_cout_{l}", [c.ND, P, c.D]),
                (f"d_ff1_{l}", [c.NFF, P, c.D]), (f"d_ff2_{l}", [c.NFF, P, c.D])]:
            w[nm] = din(nm, sh)
    w["head_t"] = din("head_t", [c.Q, P, c.V])

    opt = {}
    for nm in c.flags:
        if "_qkv_b_" in nm or "_sqkv_b_" in nm or "_cqkv_b_" in nm:
            opt[nm] = din(nm, [P, 2 * c.ND], F32)   # q,k bias cols per chunk
        elif "_ff1_b_" in nm:
            opt[nm] = din(nm, [P, c.NFF], F32)
        elif nm == "head_b":
            opt[nm] = din(nm, [P, c.Q * c.V], F32)
        else:
            opt[nm] = din(nm, [P, c.D], F32)

    logits = nc.dram_tensor("logits", [c.Q, c.TL, c.V], F32,
                            kind="ExternalOutput")

    groups = [[g * c.SP + i for i in range(c.SP)]
              for g in range(c.n_cores // c.SP)]

    with tile.TileContext(nc) as tc:
        _emit(nc, tc, c, x_emb_in, y_emb_in, rridx, gates,
              w, opt, logits, groups, WDT)
    nc.compile()
    return nc


def _emit(nc, tc, c, x_emb_in, y_emb_in, rridx, gates,
          w, opt, logits, groups, WDT):
    from contextlib import ExitStack
    es = ExitStack()
    pool = lambda name, bufs, space="SBUF": es.enter_context(
        tc.tile_pool(name=name, bufs=bufs, space=space))

    const = pool("const", 1)
    persist = pool("persist", 1)
    wpool = pool("wpool", 5)       # lhsT-style weight chunks (qkv/ff1)
    wk = pool("wk", 7)             # rhs-style weight chunks (v/out/ff2/head)
    act = pool("act", 3)
    scratch = pool("scratch", 2)
    dram = pool("dram", 4, space="DRAM")
    ps_big = pool("ps_big", 4, space="PSUM")    # 4 banks: accum for out/ff2/v
    ps_chunk = pool("ps_chunk", 2, space="PSUM")  # 2 banks: qkv/ff1/scores/tr
    ps_att = pool("ps_att", 2, space="PSUM")    # 2 banks: AV accum

    def wdma(i, out, in_, noscalar=False):
        """Weight-stream DMA spread over the two HWDGE queues.  The gpsimd
        SWDGE queue is reserved for AllGather receives so they never queue
        behind weight traffic."""
        engs = [nc.sync, nc.scalar]
        engs[i % 2].dma_start(out=out, in_=in_)

    ISC = 1.0 / 64.0 if c.FP8 else None   # inverse weight prescale

    def evac(dsl, psl, eng, bias=None):
        """PSUM -> SBUF copy with optional 1/WSCALE and bias fold."""
        if ISC is None:
            if bias is not None:
                nc.vector.tensor_scalar(out=dsl, in0=psl, scalar1=bias,
                                        scalar2=None, op0=ALU.add)
            elif eng == "s":
                nc.scalar.copy(out=dsl, in_=psl)
            else:
                nc.vector.tensor_copy(out=dsl, in_=psl)
        else:
            if bias is not None:
                nc.vector.tensor_scalar(out=dsl, in0=psl, scalar1=ISC,
                                        scalar2=bias, op0=ALU.mult,
                                        op1=ALU.add)
            elif eng == "s":
                nc.scalar.activation(dsl, psl, AF.Copy, scale=ISC)
            else:
                nc.vector.tensor_scalar_mul(dsl, psl, ISC)

    def big_tile():
        return ps_big.tile([P, 512], F32, tag="ps_big", name="psb")

    def chunk_tile():
        t = ps_chunk.tile([P, 512], F32, tag="ps_chunk", name="psc")
        return t

    def att_tile():
        return ps_att.tile([P, 512], F32, tag="ps_att", name="psa")

    # constants
    ident_f = const.tile([P, P], F32, name="ident_f")
    make_identity(nc, ident_f[:])
    causT = const.tile([P, P], F32, name="causT")
    nc.gpsimd.memset(causT[:], 0.0)
    # transposed causal: fill -1e9 where tk > tq  (iota = tq - tk >= 0 keeps)
    nc.gpsimd.affine_select(out=causT[:], in_=causT[:],
                            compare_op=ALU.is_ge, fill=-1e9, base=0,
                            pattern=[[1, P]], channel_multiplier=-1)
    eps_t = const.tile([P, 1], F32, name="eps_t")
    nc.vector.memset(eps_t[:], 1e-5)
    gates_sb = const.tile([P, 4], F32, name="gates_sb")
    nc.sync.dma_start(out=gates_sb[:], in_=gates[:])
    rridx_sb = const.tile([P, 1], I32, name="rridx_sb")
    nc.sync.dma_start(out=rridx_sb[:], in_=rridx[:])

    opt_sb = {}
    for nm in opt:
        shp = list(opt[nm].shape)
        t = const.tile(shp, F32, name=f"sb_{nm}")
        nc.sync.dma_start(out=t[:], in_=opt[nm][:])
        opt_sb[nm] = t

    # persistent activations
    x_nat = [persist.tile([P, c.D], F32, name=f"x_{t}", tag=f"x_{t}")
             for t in range(c.NTL)]
    y_emb = [persist.tile([P, c.D], F32, name=f"y_{t}", tag=f"y_{t}")
             for t in range(c.NTL)]
    ADT = F8 if c.FP8 else BF        # fat-matmul activation dtype
    x_T = persist.tile([P, c.ND * c.TL], ADT, name="x_T", tag="x_T")
    mem_T = persist.tile([P, c.ND * c.TL], ADT, name="mem_T", tag="mem_T")
    q_T = persist.tile([P, c.ND * c.TL], BF, name="q_T", tag="q_T")
    # combined (K^T, V_aug) tiles: [own half 6144 | peer half 6144]; each
    # half is [k slot, k slot, v slot, v slot] so the AG send/recv are single
    # contiguous row blocks.
    kv = persist.tile([P, 2 * c.AGW], BF, name="kv", tag="kv")
    ckv = [persist.tile([P, 2 * c.AGW], BF, name=f"ckv{i}", tag=f"ckv{i}")
           for i in range(2)]
    attn_T = persist.tile([P, c.ND * c.TL], ADT, name="attn_T", tag="attn_T")
    h_T = persist.tile([P, c.NFF * c.TL], ADT, name="h_T", tag="h_T")

    # ones columns of the v_aug blocks (set once; projections/recv write the
    # value cols, and the AG transports the peer's ones columns verbatim)
    for kvt in [kv] + ckv:
        for s in range(c.NT):
            blk = kvt[:, c.vcol(s):c.vcol(s) + c.WA].rearrange(
                "p (h f) -> p h f", f=P)
            nc.vector.memset(blk[:, :, 64:128], 1.0)

    # ---------------- helpers ----------------
    def transpose_tile(dst, t):
        """dst[:, d*TL + t*P : +P] = x_nat[t][:, d*P:+P].T (fp32 cast)."""
        for d in range(c.ND):
            pt = ps_chunk.tile([P, P], F32, tag="ps_chunk", name="pt")
            nc.tensor.transpose(pt[:], x_nat[t][:, d * P:(d + 1) * P],
                                ident_f[:])
            dsl = dst[:, d * c.TL + t * P: d * c.TL + t * P + P]
            if (t + d) % 2 == 0:
                nc.scalar.copy(out=dsl, in_=pt[:])
            else:
                nc.vector.tensor_copy(out=dsl, in_=pt[:])

    def transpose_x_into(dst):
        for t in range(c.NTL):
            transpose_tile(dst, t)

    def embed(src_dram, dst):
        for t in range(c.NTL):
            nc.sync.dma_start(out=dst[t][:],
                              in_=src_dram[t * P:(t + 1) * P, :])

    def ln_tile(t, sub, gname, bname):
        """x_nat[t] = LN(x_nat[t] + sub) * g + b (post-norm)."""
        xt = x_nat[t]
        s1 = scratch.tile([P, 1], F32, tag="lnstat", name="s1", bufs=8)
        s2 = scratch.tile([P, 1], F32, tag="lnstat", name="s2", bufs=8)
        sq = scratch.tile([P, c.D], BF, tag="lnsq", name="sq")
        nc.vector.tensor_tensor(out=xt[:], in0=xt[:], in1=sub[:], op=ALU.add)
        nc.vector.reduce_sum(out=s1[:], in_=xt[:], axis=AX.X)
        nc.scalar.activation(sq[:], xt[:], AF.Square, accum_out=s2[:])
        mean = scratch.tile([P, 1], F32, tag="lnstat", name="mean", bufs=8)
        var = scratch.tile([P, 1], F32, tag="lnstat", name="var", bufs=8)
        m2 = scratch.tile([P, 1], F32, tag="lnstat", name="m2", bufs=8)
        nc.vector.tensor_scalar_mul(mean[:], s1[:], 1.0 / c.D)
        nc.vector.tensor_scalar_mul(var[:], s2[:], 1.0 / c.D)
        nc.vector.tensor_tensor(out=m2[:], in0=mean[:], in1=mean[:],
                                op=ALU.mult)
        nc.vector.tensor_tensor(out=var[:], in0=var[:], in1=m2[:],
                                op=ALU.subtract)
        rstd = scratch.tile([P, 1], F32, tag="lnstat", name="rstd", bufs=8)
        nc.scalar.activation(rstd[:], var[:], AF.Sqrt, bias=eps_t[:])
        nc.vector.reciprocal(rstd[:], rstd[:])
        nc.vector.tensor_scalar(out=xt[:], in0=xt[:], scalar1=mean[:],
                                scalar2=rstd[:], op0=ALU.subtract,
                                op1=ALU.mult)
        if gname in opt_sb:
            nc.vector.tensor_tensor(out=xt[:], in0=xt[:],
                                    in1=opt_sb[gname][:], op=ALU.mult)
        if bname in opt_sb:
            nc.vector.tensor_tensor(out=xt[:], in0=xt[:],
                                    in1=opt_sb[bname][:], op=ALU.add)

    def proj_qk(wname, bname, src_T, dst, kind):
        """kind='q': chunks 0-7 -> dst[:, m*TL cols]; kind='k': chunks 8-15
        -> the own key slots of the combined kv tile `dst`."""
        cbase = 0 if kind == "q" else c.ND
        for m in range(c.ND):
            wt = wpool.tile([P, c.D], WDT, tag="wqkv", name="wt")
            wdma(m, wt[:], w[wname][cbase + m], noscalar=True)
            ps = chunk_tile()[:, 0:c.TL]
            if c.FP8:
                wt3 = wt[:].rearrange("p (k m) -> p k m", m=P)
                x3 = src_T[:].rearrange("p (k t) -> p k t", t=c.TL)
                for j in range(c.ND // 2):
                    nc.tensor.matmul(ps[:], wt3[:, 2 * j:2 * j + 2, :],
                                     x3[:, 2 * j:2 * j + 2, :],
                                     start=(j == 0), stop=(j == c.ND // 2 - 1),
                                     perf_mode=DR)
            else:
                for k in range(c.ND):
                    nc.tensor.matmul(ps[:], wt[:, k * P:(k + 1) * P],
                                     src_T[:, k * c.TL:(k + 1) * c.TL],
                                     start=(k == 0), stop=(k == c.ND - 1))
            if bname in opt_sb:
                bcol = m if kind == "q" else c.ND + m
                bias = opt_sb[bname][:, bcol:bcol + 1]
            else:
                bias = None
            if kind == "q":
                evac(dst[:, m * c.TL:(m + 1) * c.TL], ps[:],
                     "v" if m % 2 == 0 else "s", bias)
            else:
                for tl in range(c.NTL):
                    dsl = dst[:, c.kcol(tl, m):c.kcol(tl, m) + P]
                    psl = ps[:, tl * P:(tl + 1) * P]
                    evac(dsl, psl, "v", bias)

    def proj_v(wname, src_T, dst_aug):
        """chunks 16-23 (rhs layout): natural v for own tiles -> dst_aug
        slots 0..NTL-1 data columns."""
        NB = c.D // 512
        pss = [[big_tile() for n in range(NB)] for t in range(c.NTL)]
        if c.FP8:
            src3 = src_T[:].rearrange("p (k t) -> p k t", t=c.TL)
            for j in range(c.ND // 2):
                wt = wk.tile([P, 2 * c.D], F8, tag="wv", name="wt")
                wt3 = wt[:].rearrange("p (k d) -> p k d", d=c.D)
                wdma(j, wt3,
                     w[wname][2 * c.ND + 2 * j:2 * c.ND + 2 * j + 2]
                     .rearrange("k p d -> p k d"))
                for t in range(c.NTL):
                    for n in range(NB):
                        nc.tensor.matmul(
                            pss[t][n][:],
                            src3[:, 2 * j:2 * j + 2, t * P:t * P + P],
                            wt3[:, :, n * 512:(n + 1) * 512],
                            start=(j == 0), stop=(j == c.ND // 2 - 1),
                            perf_mode=DR)
        else:
            for k in range(c.ND):
                wt = wk.tile([P, c.D], BF, tag="wv", name="wt")
                wdma(k, wt[:], w[wname][2 * c.ND + k], noscalar=True)
                for t in range(c.NTL):
                    for n in range(NB):
                        nc.tensor.matmul(
                            pss[t][n][:],
                            src_T[:, k * c.TL + t * P: k * c.TL + t * P + P],
                            wt[:, n * 512:(n + 1) * 512],
                            start=(k == 0), stop=(k == c.ND - 1))
        for t in range(c.NTL):
            blk = dst_aug[:, c.vcol(t):c.vcol(t) + c.WA].rearrange(
                "p (h f) -> p h f", f=P)
            for n in range(NB):
                # 512 cols = 8 heads' worth of 64-wide value blocks
                psv = pss[t][n][:].rearrange("p (h f) -> p h f", f=64)
                dstb = blk[:, n * 8:(n + 1) * 8, 0:64]
                if c.FP8:
                    nc.scalar.activation(dstb, psv, AF.Copy, scale=1.0 / 64.0)
                else:
                    nc.scalar.copy(out=dstb, in_=psv)

    KR = c.NTL * c.KSLOT            # k region width per half (2048)
    VR = c.AGW - KR

    def ag_start(kv_dst, lo, hi, tag):
        """Send + AllGather-trigger for kv_dst[:, lo:hi].  The receive is a
        SEPARATE call so triggers never queue behind earlier receives on the
        in-order gpsimd stream."""
        agin = dram.tile([P, hi - lo], BF, tag=f"agin{tag}", name="agin")
        agout = dram.tile([c.SP * P, hi - lo], BF, tag=f"agout{tag}",
                          name="agout")
        with tc.high_priority():
            nc.sync.dma_start(out=agin[:], in_=kv_dst[:, lo:hi])
        nc.gpsimd.collective_compute(
            "AllGather", ALU.bypass, replica_groups=groups,
            ins=[agin[:].opt()], outs=[agout[:].opt()])
        return (agout, kv_dst, c.AGW + lo)

    def ag_recv(h):
        agout, kv_dst, dst0 = h
        n = agout.shape[-1]
        nc.gpsimd.indirect_dma_start(
            out=kv_dst[:, dst0:dst0 + n],
            out_offset=None,
            in_=agout[:],
            in_offset=IndirectOffsetOnAxis(ap=rridx_sb[:, :1], axis=0))

    def ag_k_start(kv_dst):
        return ag_start(kv_dst, 0, KR, "k")

    def ag_v_start(kv_dst):
        return ag_start(kv_dst, KR, c.AGW, "v")

    def attention(kvt, mode):
        """q_T x kvt -> attn_T.  mode: 'full' (all 4 slots, no mask) or
        'causal' (slot structure for decoder self-attention)."""
        for h in range(c.H):
            m = h // 2
            po = 64 * (h % 2)
            # score slot pairs share one PSUM bank + one exp where possible:
            # pair block cols [s%2 * TL : +TL]; in causal mode slot 1 only
            # has q cols [P:TL] (placed at [TL+P:2TL]) and slots 0/1 get the
            # diagonal causT mask; slots 2/3 share one gate bias.
            at = []
            for pi, (s0, s1) in enumerate([(0, 1), (2, 3)]):
                pss = big_tile()
                a = act.tile([P, 2 * c.TL], BF, tag=f"ATp{pi}", name="at",
                             bufs=3)
                qr = []
                for si, s in enumerate((s0, s1)):
                    q0 = P if (mode == "causal" and s == 1) else 0
                    base = si * c.TL
                    nc.tensor.matmul(
                        pss[:, base + q0: base + c.TL],
                        kvt[po:po + 64, c.kcol(s, m):c.kcol(s, m) + P],
                        q_T[po:po + 64, m * c.TL + q0: m * c.TL + c.TL],
                        start=True, stop=True)
                    if mode == "causal" and s < 2:
                        # diagonal block: q columns s*P of this slot
                        d0 = base + s * P
                        nc.vector.tensor_tensor(
                            out=pss[:, d0:d0 + P], in0=pss[:, d0:d0 + P],
                            in1=causT[:], op=ALU.add)
                    qr.append(q0)
                if mode == "causal" and pi == 1:
                    nc.scalar.activation(a[:], pss[:], AF.Exp, scale=c.SCALE,
                                         bias=gates_sb[:, s0:s0 + 1])
                elif mode == "causal":
                    # two exps: slot0 full, slot1 partial (skip the dead gap)
                    nc.scalar.activation(a[:, 0:c.TL], pss[:, 0:c.TL],
                                         AF.Exp, scale=c.SCALE)
                    nc.scalar.activation(a[:, c.TL + P:2 * c.TL],
                                         pss[:, c.TL + P:2 * c.TL],
                                         AF.Exp, scale=c.SCALE)
                else:
                    nc.scalar.activation(a[:], pss[:], AF.Exp, scale=c.SCALE)
                at.append((s0, a, 0, qr[0]))
                at.append((s1, a, c.TL, qr[1]))
            ps_o = att_tile()[:, 0:c.TL]
            for i, (s, a, base, q0) in enumerate(at):
                nc.tensor.matmul(
                    ps_o[:, q0:c.TL],
                    kvt[:, c.vcol(s) + h * P: c.vcol(s) + (h + 1) * P],
                    a[:, base + q0: base + c.TL],
                    start=(i == 0), stop=(i == len(at) - 1))
            rden = scratch.tile([64, c.TL], F32, tag="rden", name="rden",
                                bufs=2)
            nc.vector.reciprocal(rden[:], ps_o[64:128, :])
            nc.vector.tensor_tensor(
                out=attn_T[po:po + 64, m * c.TL:(m + 1) * c.TL],
                in0=ps_o[0:64, :], in1=rden[:], op=ALU.mult)

    def mm_to_natural(src_T, nk, wname, bname, noscalar=False):
        """[TL, D] = src_T.T @ W (k-chunk streaming, PSUM accumulate),
        returned as per-t bf16 [P, D] tiles."""
        NB = c.D // 512
        pss = [[big_tile() for n in range(NB)] for t in range(c.NTL)]
        if c.FP8:
            src3 = src_T[:].rearrange("p (k t) -> p k t", t=c.TL)
            for j in range(nk // 2):
                wt = wk.tile([P, 2 * c.D], F8, tag="wnat", name="wt")
                wt3 = wt[:].rearrange("p (k d) -> p k d", d=c.D)
                wdma(j, wt3,
                     w[wname][2 * j:2 * j + 2].rearrange("k p d -> p k d"))
                for t in range(c.NTL):
                    for n in range(NB):
                        nc.tensor.matmul(
                            pss[t][n][:],
                            src3[:, 2 * j:2 * j + 2, t * P:t * P + P],
                            wt3[:, :, n * 512:(n + 1) * 512],
                            start=(j == 0), stop=(j == nk // 2 - 1),
                            perf_mode=DR)
        else:
            for k in range(nk):
                wt = wk.tile([P, c.D], BF, tag="wnat", name="wt")
                wdma(k, wt[:], w[wname][k], noscalar=noscalar)
                for t in range(c.NTL):
                    for n in range(NB):
                        nc.tensor.matmul(
                            pss[t][n][:],
                            src_T[:, k * c.TL + t * P: k * c.TL + t * P + P],
                            wt[:, n * 512:(n + 1) * 512],
                            start=(k == 0), stop=(k == nk - 1))
        parts = []
        for t in range(c.NTL):
            sb = scratch.tile([P, c.D], BF, tag="oproj", name="sb", bufs=3)
            for n in range(NB):
                evac(sb[:, n * 512:(n + 1) * 512], pss[t][n][:],
                     "s" if (t + n) % 2 == 0 else "v")
            if bname in opt_sb:
                nc.vector.tensor_tensor(out=sb[:], in0=sb[:],
                                        in1=opt_sb[bname][:], op=ALU.add)
            parts.append(sb)
        return parts

    def ffn(w1name, b1name, w2name, b2name):
        x3 = x_T[:].rearrange("p (k t) -> p k t", t=c.TL)
        for mchunk in range(c.NFF):
            wt = wpool.tile([P, c.D], WDT, tag="wff1", name="wt")
            wdma(mchunk, wt[:], w[w1name][mchunk])
            ps = chunk_tile()[:, 0:c.TL]
            if c.FP8:
                wt3 = wt[:].rearrange("p (k m) -> p k m", m=P)
                for j in range(c.ND // 2):
                    nc.tensor.matmul(ps[:], wt3[:, 2 * j:2 * j + 2, :],
                                     x3[:, 2 * j:2 * j + 2, :],
                                     start=(j == 0), stop=(j == c.ND // 2 - 1),
                                     perf_mode=DR)
            else:
                for k in range(c.ND):
                    nc.tensor.matmul(ps[:], wt[:, k * P:(k + 1) * P],
                                     x_T[:, k * c.TL:(k + 1) * c.TL],
                                     start=(k == 0), stop=(k == c.ND - 1))
            dsl = h_T[:, mchunk * c.TL:(mchunk + 1) * c.TL]
            kw = {"scale": 1.0 / 64.0} if c.FP8 else {}
            if b1name in opt_sb:
                nc.scalar.activation(dsl, ps[:], AF.Relu,
                                     bias=opt_sb[b1name][:, mchunk:mchunk + 1],
                                     **kw)
            else:
                nc.scalar.activation(dsl, ps[:], AF.Relu, **kw)
        return mm_to_natural(h_T, c.NFF, w2name, b2name)

    def ag_warm(dep_tile):
        """Tiny AllGather that wakes the cc stream ahead of a real AG;
        dep_tile pins its position in the schedule."""
        n = dep_tile.shape[-1]
        win = dram.tile([P, n], dep_tile.dtype, tag="warmin", name="win")
        wout = dram.tile([c.SP * P, n], dep_tile.dtype, tag="warmout",
                         name="wout")
        nc.sync.dma_start(out=win[:], in_=dep_tile)
        nc.gpsimd.collective_compute(
            "AllGather", ALU.bypass, replica_groups=groups,
            ins=[win[:].opt()], outs=[wout[:].opt()])

    def cross_kv_proj(l):
        buf = l % 2
        proj_qk(f"d_cqkv_{l}", f"d_cqkv_b_{l}", mem_T, ckv[buf], "k")
        proj_v(f"d_cqkv_{l}", mem_T, ckv[buf])

    def cross_kv_prefetch(l):
        """Project + AllGather cross-attention K/V for decoder layer l."""
        cross_kv_proj(l)
        hk = ag_k_start(ckv[l % 2])
        hv = ag_v_start(ckv[l % 2])
        ag_recv(hk)
        ag_recv(hv)

    # ---------------- encoder ----------------
    ag_warm(gates_sb[:, 0:4])
    embed(x_emb_in, x_nat)
    embed(y_emb_in, y_emb)
    transpose_x_into(x_T)
    for l in range(c.L):
        proj_qk(f"e_qkv_{l}", f"e_qkv_b_{l}", x_T, kv, "k")
        if l > 0:
            ag_warm(kv[:, 0:8])
        hk = ag_k_start(kv)
        proj_v(f"e_qkv_{l}", x_T, kv)
        hv = ag_v_start(kv)
        ag_recv(hk)
        ag_recv(hv)
        proj_qk(f"e_qkv_{l}", f"e_qkv_b_{l}", x_T, q_T, "q")
        attention(kv, "full")
        parts = mm_to_natural(attn_T, c.ND, f"e_out_{l}", f"e_out_b_{l}",
                              noscalar=True)
        for t in range(c.NTL):
            ln_tile(t, parts[t], f"e_ln1_w_{l}", f"e_ln1_b_{l}")
            transpose_tile(x_T, t)
        parts = ffn(f"e_ff1_{l}", f"e_ff1_b_{l}", f"e_ff2_{l}", f"e_ff2_b_{l}")
        last = l == c.L - 1
        for t in range(c.NTL):
            ln_tile(t, parts[t], f"e_ln2_w_{l}", f"e_ln2_b_{l}")
            transpose_tile(mem_T if last else x_T, t)

    # cross K/V for decoder layer 0 (hides under decoder embed + self attn)
    cross_kv_prefetch(0)

    # ---------------- decoder ----------------
    for t in range(c.NTL):
        nc.vector.tensor_copy(out=x_nat[t][:], in_=y_emb[t][:])
        transpose_tile(x_T, t)
    for l in range(c.L):
        proj_qk(f"d_sqkv_{l}", f"d_sqkv_b_{l}", x_T, kv, "k")
        ag_warm(kv[:, 0:8])
        hk = ag_k_start(kv)
        proj_v(f"d_sqkv_{l}", x_T, kv)
        hv = ag_v_start(kv)
        # cross K/V for the NEXT layer: the projections are PE cover work for
        # the self AG flight; their triggers ride the warm cc stream and the
        # receives come after the self receives
        hck = hcv = None
        if l + 1 < c.L:
            cross_kv_proj(l + 1)
            hck = ag_k_start(ckv[(l + 1) % 2])
            hcv = ag_v_start(ckv[(l + 1) % 2])
        ag_recv(hk)
        ag_recv(hv)
        if hck is not None:
            ag_recv(hck)
            ag_recv(hcv)
        proj_qk(f"d_sqkv_{l}", f"d_sqkv_b_{l}", x_T, q_T, "q")
        attention(kv, "causal")
        parts = mm_to_natural(attn_T, c.ND, f"d_sout_{l}", f"d_sout_b_{l}",
                              noscalar=True)
        for t in range(c.NTL):
            ln_tile(t, parts[t], f"d_ln1_w_{l}", f"d_ln1_b_{l}")
            transpose_tile(x_T, t)
        proj_qk(f"d_cqkv_{l}", f"d_cqkv_b_{l}", x_T, q_T, "q")
        attention(ckv[l % 2], "full")
        parts = mm_to_natural(attn_T, c.ND, f"d_cout_{l}", f"d_cout_b_{l}",
                              noscalar=True)
        for t in range(c.NTL):
            ln_tile(t, parts[t], f"d_ln2_w_{l}", f"d_ln2_b_{l}")
            transpose_tile(x_T, t)
        parts = ffn(f"d_ff1_{l}", f"d_ff1_b_{l}", f"d_ff2_{l}", f"d_ff2_b_{l}")
        for t in range(c.NTL):
            ln_tile(t, parts[t], f"d_ln3_w_{l}", f"d_ln3_b_{l}")
            transpose_tile(x_T, t)

    # ---------------- output head ----------------
    NBV = c.V // 512
    for j in range(c.Q):
        hw = wk.tile([P, c.V], WDT, tag="whead", name="hw", bufs=2)
        wdma(j, hw[:], w["head_t"][j])
        for t in range(c.NTL):
            sb = scratch.tile([P, c.V], F32, tag="lgt", name="sb", bufs=3)
            for n in range(NBV):
                ps = big_tile()
                nc.tensor.matmul(
                    ps[:], x_T[:, j * c.TL + t * P: j * c.TL + t * P + P],
                    hw[:, n * 512:(n + 1) * 512], start=True, stop=True)
                evac(sb[:, n * 512:(n + 1) * 512], ps[:], "s")
            if "head_b" in opt_sb:
                nc.vector.tensor_tensor(
                    out=sb[:], in0=sb[:],
                    in1=opt_sb["head_b"][:, j * c.V:(j + 1) * c.V],
                    op=ALU.add)
            nc.sync.dma_start(out=logits[j, t * P:(t + 1) * P, :], in_=sb[:])

    es.close()


# --------------------------------------------------------------------------
# host side
# --------------------------------------------------------------------------

_PROG_CACHE = {}


def parse_cfg(inputs, n_cores=8, fp8=None):
    B, Q, T = inputs["input_codes"].shape
    _, V, E = np.asarray(inputs["tok_emb"]).shape
    L, _, D = np.asarray(inputs["e_qkv_w"]).shape
    FF = np.asarray(inputs["e_ff1_w"]).shape[1]
    H = D // 64
    flags = set()
    for l in range(L):
        for knm in ["e_qkv_b", "d_sqkv_b", "d_cqkv_b", "e_ff1_b", "d_ff1_b",
                    "e_out_b", "e_ff2_b", "d_sout_b", "d_cout_b", "d_ff2_b"]:
            if np.any(np.asarray(inputs[knm])[l]):
                flags.add(f"{knm}_{l}")
        for ln in ["e_ln1", "e_ln2", "d_ln1", "d_ln2", "d_ln3"]:
            if not np.all(np.asarray(inputs[ln + "_w"])[l] == 1.0):
                flags.add(f"{ln}_w_{l}")
            if np.any(np.asarray(inputs[ln + "_b"])[l]):
                flags.add(f"{ln}_b_{l}")
    if np.any(np.asarray(inputs["head_b"])):
        flags.add("head_b")
    if fp8 is None:
        fp8 = os.environ.get("BASS_S2S_FP8", "0") == "1"
    # v-bias unsupported in-kernel; fall back assertion
    for l in range(L):
        for nm in ["e_qkv_b", "d_sqkv_b", "d_cqkv_b"]:
            vb = np.asarray(inputs[nm])[l][2 * D:3 * D]
            assert not np.any(vb), "nonzero v bias not supported"
    return Cfg(B, Q, T, D, H, V, L, FF, n_cores, flags, fp8=fp8)


def _lhsT_chunks(wm, D):
    """[M, D] row-major weight -> [M//128, 128(p=in%128), ...] lhsT chunk
    layout: chunk c element [p, nd*128 + m] = wm[c*128 + m, nd*128 + p]."""
    M = wm.shape[0]
    nd = D // P
    out = np.empty((M // P, P, D), np.float32)
    for cc in range(M // P):
        wc = wm[cc * P:(cc + 1) * P, :]          # [128 m, D in]
        out[cc] = wc.T.reshape(nd, P, P).transpose(1, 0, 2).reshape(P, D)
    return out


def _rhs_chunks(wm, D_out):
    """[D_out, K] row-major weight -> [K//128, 128(p=k%128), D_out] rhs
    chunk layout: chunk k element [p, n] = wm[n, k*128 + p]."""
    K = wm.shape[1]
    return np.ascontiguousarray(
        wm.T.reshape(K // P, P, D_out))


def build_inmaps(inputs, c: Cfg):
    g = lambda nm: np.asarray(inputs[nm], np.float32)
    if c.FP8:
        def bf(a):
            a = np.ascontiguousarray(a, dtype=np.float32) * c.WSCALE
            return np.clip(a, -240.0, 240.0).astype(FP8NP)
    else:
        bf = lambda a: np.ascontiguousarray(a, dtype=np.float32).astype(BF16)

    tok = np.asarray(inputs["tok_emb"], np.float32)
    posf = np.ascontiguousarray(g("pos_emb")[0, :c.T, :])
    head_w = g("head_w")

    common = {}
    head_t = np.stack([head_w[q].T for q in range(c.Q)])    # [Q, E, V]
    common["head_t"] = bf(head_t)
    if "head_b" in c.flags:
        hb = g("head_b").reshape(-1)
        common["head_b"] = np.broadcast_to(hb, (P, c.Q * c.V)).copy()

    for pre, wq, wo in [("e_qkv", "e_qkv_w", None), ("e_out", None, "e_out_w"),
                        ("d_sqkv", "d_sqkv_w", None),
                        ("d_sout", None, "d_sout_w"),
                        ("d_cqkv", "d_cqkv_w", None),
                        ("d_cout", None, "d_cout_w")]:
        for l in range(c.L):
            if wq is not None:
                qkv = g(wq)[l]                    # [3D, D]
                qk = _lhsT_chunks(qkv[0:2 * c.D], c.D)      # q,k chunks
                vv = _rhs_chunks(qkv[2 * c.D:3 * c.D], c.D)
                common[f"{pre}_{l}"] = bf(np.concatenate([qk, vv], axis=0))
            else:
                wo_l = g(wo)[l]                   # [D, D] rows = out dim
                common[f"{pre}_{l}"] = bf(_rhs_chunks(wo_l, c.D))
    for l in range(c.L):
        common[f"e_ff1_{l}"] = bf(_lhsT_chunks(g("e_ff1_w")[l], c.D))
        common[f"d_ff1_{l}"] = bf(_lhsT_chunks(g("d_ff1_w")[l], c.D))
        common[f"e_ff2_{l}"] = bf(_rhs_chunks(g("e_ff2_w")[l], c.D))
        common[f"d_ff2_{l}"] = bf(_rhs_chunks(g("d_ff2_w")[l], c.D))

    # optional biases
    for l in range(c.L):
        for knm in ["e_qkv_b", "d_sqkv_b", "d_cqkv_b"]:
            if f"{knm}_{l}" in c.flags:
                b = g(knm)[l][0:2 * c.D]           # q,k bias only
                common[f"{knm}_{l}"] = np.ascontiguousarray(
                    b.reshape(2 * c.ND, P).T)
        for knm in ["e_ff1_b", "d_ff1_b"]:
            if f"{knm}_{l}" in c.flags:
                common[f"{knm}_{l}"] = np.ascontiguousarray(
                    g(knm)[l].reshape(c.NFF, P).T)
        for knm in ["e_out_b", "e_ff2_b", "d_sout_b", "d_cout_b", "d_ff2_b"]:
            if f"{knm}_{l}" in c.flags:
                common[f"{knm}_{l}"] = np.broadcast_to(
                    g(knm)[l], (P, c.D)).copy()
        for ln in ["e_ln1", "e_ln2", "d_ln1", "d_ln2", "d_ln3"]:
            for sfx in ["w", "b"]:
                if f"{ln}_{sfx}_{l}" in c.flags:
                    common[f"{ln}_{sfx}_{l}"] = np.broadcast_to(
                        g(f"{ln}_{sfx}")[l], (P, c.D)).copy()

    codes_in = np.asarray(inputs["input_codes"], np.int32)
    codes_tgt = np.asarray(inputs["target_codes"], np.int32)

    def embed_host(codes_bqt):
        # [Q, T] codes -> [T, D] embedding (concat per-quantizer) + pos
        e = np.concatenate([tok[q][codes_bqt[q]] for q in range(c.Q)],
                           axis=-1)
        return e + posf

    emb_in = [embed_host(codes_in[b]) for b in range(c.B)]
    emb_tgt = [embed_host(codes_tgt[b]) for b in range(c.B)]
    in_maps = []
    for core in range(c.n_cores):
        b, h = core // c.SP, core % c.SP
        m = dict(common)
        sl = slice(h * c.TL, (h + 1) * c.TL)
        m["x_emb"] = np.ascontiguousarray(emb_in[b % c.B][sl])
        m["y_emb"] = np.ascontiguousarray(emb_tgt[b % c.B][sl])
        m["rridx"] = ((1 - h) * P + np.arange(P, dtype=np.int32)
                      ).reshape(P, 1)
        gate = np.zeros((P, 4), np.float32)
        if h == 0:
            gate[:, 2] = -1e9
            gate[:, 3] = -1e9
        m["gates"] = gate
        in_maps.append(m)
    return in_maps


def postprocess(results, c: Cfg):
    out = np.empty((c.B, c.T, c.Q, c.V), np.float32)
    for b in range(c.B):
        for h in range(c.SP):
            r = results[b * c.SP + h]["logits"]      # [Q, TL, V]
            out[b, h * c.TL:(h + 1) * c.TL] = r.transpose(1, 0, 2)
    return out


def run(inputs, trace=False):
    from concourse.bass_utils import run_bass_kernel_spmd
    c = parse_cfg(inputs)
    key = c.key()
    if key not in _PROG_CACHE:
        _PROG_CACHE[key] = build_program(c)
    nc = _PROG_CACHE[key]
    in_maps = build_inmaps(inputs, c)
    res = run_bass_kernel_spmd(nc, in_maps, list(range(c.n_cores)),
                               trace=trace)
    return postprocess(res.results, c), res


def kernel(**inputs):
    out, _ = run(inputs, trace=False)
    return out
